# revision 9
# baseline (speedup 1.0000x reference)
"""Adaptive LM head (3-tier chunked softmax cross-entropy) on 8 TRN2 NeuronCores.

Strategy: data-parallel over B_T = 8192 rows (1024 rows/core; weights
replicated). Per core:
  - logits for each tier computed as bf16 matmuls (hT stationary, W streaming,
    f32->bf16 cast done in-flight by SWDGE DMA), PSUM accumulation over the
    contraction dim, 2048-wide vocab super-tiles.
  - ScalarE activation(Exp, accum_out=...) fuses exp + per-row sum in a single
    pass over each [128, 2048] logit tile (accum_out -> per-row partial Z).
  - tier units are interleaved (PE-heavy tier0 against ACT-heavy tier2) so
    TensorE stays dense/warm while ScalarE drains exp sums.
  - target logit = dot(feature_row, W[:, target]) computed exactly in f32:
    indirect-DMA gather of transposed-weight rows + fused scalar_tensor_tensor
    multiply-reduce, spread across the main loop.
  - per-core partial loss (sum_rows(log Z - target_logit)/8192) is the output;
    the host sums the 8 partials (the unshard step for a DP loss).
"""

import numpy as np

from concourse import bacc, bass, mybir
from concourse.bass import IndirectOffsetOnAxis
from concourse.bass_utils import run_bass_kernel_spmd
from concourse.tile import TileContext

F32 = mybir.dt.float32
BF16 = mybir.dt.bfloat16
I32 = mybir.dt.int32
ALU = mybir.AluOpType
ACTF = mybir.ActivationFunctionType

P = 128
D = 1024
N_CORES = 8
RPC = 1024          # rows per core
NRT = RPC // P      # row tiles per core = 8
ST = 2048           # vocab super-tile width
NB = 512            # matmul free-dim tile (one PSUM bank)
NBM = 512           # main-loop moving-operand tile
V0, V1, V2 = 8192, 16384, 25681
PD1, PD2 = 256, 128
B_T = 8192

# Composite-tile schedule: each [128, 2048] PSUM tile packs four 512-col
# groups from DIFFERENT tiers (tier0 K=8 fill 1.7us, tier1 K=2 0.43us,
# tier2 K=1 0.21us) so every tile's matmul fill time roughly matches the
# ~2.0us ScalarE exp+sum drain -- the depth-2 PSUM ping-pong then keeps both
# engines dense. exp+accum sums all packed columns regardless of tier.
# windows: (tier0 st, [tier1 sts], [tier2 sts])
WINDOWS = [
    (0, [0, 1], [0, 1, 2]),
    (1, [2, 3], [3, 4, 5]),
    (2, [4, 5], [6, 7, 8]),
    (3, [6, 7], [9, 10, 11, 12]),
]

_NC_CACHE = None


def _ceil_div(a, b):
    return (a + b - 1) // b


def _build_graph():
    nc = bacc.Bacc("TRN2", target_bir_lowering=False, debug=False,
                   num_devices=N_CORES)

    ht_ext = nc.declare_dram_parameter("ht", [D, RPC], F32, isOutput=False)
    hr_ext = nc.declare_dram_parameter("hr", [RPC, D], F32, isOutput=False)
    tf_ext = nc.declare_dram_parameter("tf", [P, NRT], F32, isOutput=False)
    wp1_ext = nc.declare_dram_parameter("wp1", [D, PD1], F32, isOutput=False)
    wp2_ext = nc.declare_dram_parameter("wp2", [D, PD2], F32, isOutput=False)
    w0_ext = nc.declare_dram_parameter("w0", [D, V0], F32, isOutput=False)
    w1_ext = nc.declare_dram_parameter("w1", [PD1, V1], F32, isOutput=False)
    w2_ext = nc.declare_dram_parameter("w2", [PD2, V2], F32, isOutput=False)
    wt0_ext = nc.declare_dram_parameter("wt0", [V0, D], F32, isOutput=False)
    wt1_ext = nc.declare_dram_parameter("wt1", [V1, PD1], F32, isOutput=False)
    wt2_ext = nc.declare_dram_parameter("wt2", [V2, PD2], F32, isOutput=False)
    out_ext = nc.declare_dram_parameter("out", [1, 1], F32, isOutput=True)

    with TileContext(nc) as tc:
        with (
            tc.tile_pool(name="res", bufs=1) as res,
            tc.tile_pool(name="w0pool", bufs=2) as w0pool,
            tc.tile_pool(name="w1pool", bufs=4) as w1pool,
            tc.tile_pool(name="w2pool", bufs=6) as w2pool,
            tc.tile_pool(name="hrpool", bufs=2) as hrpool,
            tc.tile_pool(name="expool", bufs=3) as expool,
            tc.tile_pool(name="gpool", bufs=1) as gpool,
            tc.tile_pool(name="prodpool", bufs=1) as prodpool,
            tc.tile_pool(name="psum", bufs=2, space="PSUM") as psum,
        ):
            # ---------------- resident tiles ----------------
            ht_sb = res.tile([P, 8 * RPC], BF16, tag="ht")       # 8 d-chunks
            wp1_sb = res.tile([P, 8 * PD1], BF16, tag="wp1")
            wp2_sb = res.tile([P, 8 * PD2], BF16, tag="wp2")
            hp1T_sb = res.tile([P, 2 * RPC], BF16, tag="hp1T")
            hp2T_sb = res.tile([P, 1 * RPC], BF16, tag="hp2T")
            hp1r_sb = res.tile([P, NRT * PD1], F32, tag="hp1r")
            hp2r_sb = res.tile([P, NRT * PD2], F32, tag="hp2r")
            tf_sb = res.tile([P, NRT], F32, tag="tf")
            ge1 = res.tile([P, NRT], F32, tag="ge1")
            ge2 = res.tile([P, NRT], F32, tag="ge2")
            idxf = [res.tile([P, NRT], F32, tag=f"idxf{t}", name=f"idxf{t}")
                    for t in range(3)]
            idxi = [res.tile([P, NRT], I32, tag=f"idxi{t}", name=f"idxi{t}")
                    for t in range(3)]
            tl = [res.tile([P, NRT], F32, tag=f"tl{t}", name=f"tl{t}")
                  for t in range(3)]
            zbig = res.tile([P, NRT * 32], F32, tag="zbig")
            zred = res.tile([P, NRT], F32, tag="zred")
            logz = res.tile([P, NRT], F32, tag="logz")
            d1 = res.tile([P, NRT], F32, tag="d1")
            d2 = res.tile([P, NRT], F32, tag="d2")
            loss8 = res.tile([P, NRT], F32, tag="loss8")
            lossv = res.tile([P, 1], F32, tag="lossv")
            ones = res.tile([P, 1], F32, tag="ones")
            part = res.tile([1, 1], F32, tag="part")

            # ---------------- input staging ----------------
            # order matters for scheduling priority: ht (feeds everything),
            # then the first units' W slices arrive via the per-tier pools
            for k in range(8):
                nc.gpsimd.dma_start(
                    out=ht_sb[:, k * RPC:(k + 1) * RPC],
                    in_=ht_ext[k * P:(k + 1) * P, :])
            nc.sync.dma_start(out=tf_sb[:], in_=tf_ext[:, :])
            for k in range(8):
                nc.gpsimd.dma_start(
                    out=wp1_sb[:, k * PD1:(k + 1) * PD1],
                    in_=wp1_ext[k * P:(k + 1) * P, :])
                nc.gpsimd.dma_start(
                    out=wp2_sb[:, k * PD2:(k + 1) * PD2],
                    in_=wp2_ext[k * P:(k + 1) * P, :])

            nc.vector.memset(zbig[:], 0.0)
            nc.vector.memset(ones[:], 1.0)

            # ---------------- masks and in-tier indices ----------------
            nc.vector.tensor_scalar(out=ge1[:], in0=tf_sb[:], scalar1=float(V0),
                                    scalar2=None, op0=ALU.is_ge)
            nc.vector.tensor_scalar(out=ge2[:], in0=tf_sb[:],
                                    scalar1=float(V0 + V1), scalar2=None,
                                    op0=ALU.is_ge)
            nc.vector.tensor_scalar(out=idxf[0][:], in0=tf_sb[:],
                                    scalar1=float(V0 - 1), scalar2=None,
                                    op0=ALU.min)
            nc.vector.tensor_scalar(out=idxf[1][:], in0=tf_sb[:],
                                    scalar1=-float(V0), scalar2=0.0,
                                    op0=ALU.add, op1=ALU.max)
            nc.vector.tensor_scalar(out=idxf[1][:], in0=idxf[1][:],
                                    scalar1=float(V1 - 1), scalar2=None,
                                    op0=ALU.min)
            nc.vector.tensor_scalar(out=idxf[2][:], in0=tf_sb[:],
                                    scalar1=-float(V0 + V1), scalar2=0.0,
                                    op0=ALU.add, op1=ALU.max)
            nc.vector.tensor_scalar(out=idxf[2][:], in0=idxf[2][:],
                                    scalar1=float(V2 - 1), scalar2=None,
                                    op0=ALU.min)
            for t in range(3):
                nc.vector.tensor_copy(out=idxi[t][:], in_=idxf[t][:])

            # ---------------- projections ----------------
            for m in range(PD1 // P):
                for rb in range(RPC // NB):
                    ps = psum.tile([P, ST], F32, tag="ps")
                    for k in range(8):
                        nc.tensor.matmul(
                            out=ps[:, :NB],
                            lhsT=wp1_sb[:, k * PD1 + m * P: k * PD1 + (m + 1) * P],
                            rhs=ht_sb[:, k * RPC + rb * NB: k * RPC + (rb + 1) * NB],
                            start=(k == 0), stop=(k == 7))
                    nc.vector.tensor_copy(
                        out=hp1T_sb[:, m * RPC + rb * NB: m * RPC + (rb + 1) * NB],
                        in_=ps[:, :NB])
            for rb in range(RPC // NB):
                ps = psum.tile([P, ST], F32, tag="ps")
                for k in range(8):
                    nc.tensor.matmul(
                        out=ps[:, :NB],
                        lhsT=wp2_sb[:, k * PD2:(k + 1) * PD2],
                        rhs=ht_sb[:, k * RPC + rb * NB: k * RPC + (rb + 1) * NB],
                        start=(k == 0), stop=(k == 7))
                nc.vector.tensor_copy(
                    out=hp2T_sb[:, rb * NB:(rb + 1) * NB], in_=ps[:, :NB])

            for rt in range(NRT):
                ps = psum.tile([P, ST], F32, tag="ps")
                for k in range(8):
                    nc.tensor.matmul(
                        out=ps[:, :PD1],
                        lhsT=ht_sb[:, k * RPC + rt * P: k * RPC + rt * P + P],
                        rhs=wp1_sb[:, k * PD1:(k + 1) * PD1],
                        start=(k == 0), stop=(k == 7))
                nc.vector.tensor_copy(
                    out=hp1r_sb[:, rt * PD1:(rt + 1) * PD1], in_=ps[:, :PD1])
            for rt in range(NRT):
                ps = psum.tile([P, ST], F32, tag="ps")
                for k in range(8):
                    nc.tensor.matmul(
                        out=ps[:, :PD2],
                        lhsT=ht_sb[:, k * RPC + rt * P: k * RPC + rt * P + P],
                        rhs=wp2_sb[:, k * PD2:(k + 1) * PD2],
                        start=(k == 0), stop=(k == 7))
                nc.vector.tensor_copy(
                    out=hp2r_sb[:, rt * PD2:(rt + 1) * PD2], in_=ps[:, :PD2])

            # ---------------- interleaved main units ----------------
            tiers = {
                0: (V0, 8, w0_ext, ht_sb, w0pool, 8),
                1: (V1, 2, w1_ext, hp1T_sb, w1pool, 2),
                2: (V2, 1, w2_ext, hp2T_sb, w2pool, 1),
            }
            gather_src = [wt0_ext, wt1_ext, wt2_ext]
            gdim = [D, PD1, PD2]
            gmax = [V0 - 1, V1 - 1, V2 - 1]

            def emit_gather_dot(i):
                rt, t = divmod(i, 3)
                if t == 0:
                    hr_t = hrpool.tile([P, D], F32, tag="hrt", name="hrt")
                    nc.sync.dma_start(out=hr_t[:],
                                      in_=hr_ext[rt * P:(rt + 1) * P, :])
                    feat_ap = hr_t[:]
                elif t == 1:
                    feat_ap = hp1r_sb[:, rt * PD1:(rt + 1) * PD1]
                else:
                    feat_ap = hp2r_sb[:, rt * PD2:(rt + 1) * PD2]
                g = gpool.tile([P, gdim[t]], F32, tag=f"g{t}", name=f"g{t}")
                nc.gpsimd.indirect_dma_start(
                    out=g[:], out_offset=None,
                    in_=gather_src[t][:, :],
                    in_offset=IndirectOffsetOnAxis(
                        ap=idxi[t][:, rt:rt + 1], axis=0),
                    bounds_check=gmax[t], oob_is_err=False)
                prod = prodpool.tile([P, D], F32, tag="prod")
                nc.vector.scalar_tensor_tensor(
                    out=prod[:, :gdim[t]],
                    in0=feat_ap, scalar=1.0, in1=g[:],
                    op0=ALU.mult, op1=ALU.mult,
                    accum_out=tl[t][:, rt:rt + 1])

            st_wtile = {}

            def ensure_st(tier, st):
                if (tier, st) in st_wtile:
                    return
                V, K, w_ext, lhsT_sb, wpool, nchunks = tiers[tier]
                w = min(ST, V - st * ST)
                wtile = wpool.tile([P, nchunks * ST], BF16,
                                   tag=f"w{tier}", name=f"w{tier}")
                for k in range(K):
                    nc.gpsimd.dma_start(
                        out=wtile[:, k * ST: k * ST + w],
                        in_=w_ext[k * P:(k + 1) * P, st * ST: st * ST + w])
                st_wtile[(tier, st)] = wtile

            def emit_tile(groups, rt, zcol):
                # groups: list of (tier, st, g, gw); packed into one psum tile
                ps = psum.tile([P, ST], F32, tag="ps")
                off = 0
                for (tier, st, g, gw) in groups:
                    V, K, w_ext, lhsT_sb, wpool, nchunks = tiers[tier]
                    wtile = st_wtile[(tier, st)]
                    for k in range(K):
                        nc.tensor.matmul(
                            out=ps[:, off: off + gw],
                            lhsT=lhsT_sb[:, k * RPC + rt * P:
                                         k * RPC + rt * P + P],
                            rhs=wtile[:, k * ST + g * NB: k * ST + g * NB + gw],
                            start=(k == 0), stop=(k == K - 1))
                    off += gw
                ex = expool.tile([P, ST], BF16, tag="ex")
                nc.scalar.activation(
                    ex[:, :off], ps[:, :off], ACTF.Exp,
                    accum_out=zbig[:, rt * 32 + zcol: rt * 32 + zcol + 1])

            def build_tiles(As, Bs, Cs):
                tiles = []
                ia = ib = ic = 0
                while ia < len(As):
                    t = [As[ia]]; ia += 1
                    if ib < len(Bs):
                        t.append(Bs[ib]); ib += 1
                    t += Cs[ic:ic + 2]; ic += 2
                    tiles.append(t)
                while ib < len(Bs):
                    t = Bs[ib:ib + 2]; ib += 2
                    t += Cs[ic:ic + 2]; ic += 2
                    tiles.append(t)
                while ic < len(Cs):
                    t = Cs[ic:ic + 4]; ic += len(Cs[ic:ic + 4])
                    tiles.append(t)
                return tiles

            def st_groups(tier, st):
                V = tiers[tier][0]
                w = min(ST, V - st * ST)
                gs = []
                for g in range(_ceil_div(w, NB)):
                    gs.append((tier, st, g, min(NB, w - g * NB)))
                return gs

            zcols = [0] * NRT
            blk = 0
            for (a_st, b_sts, c_sts) in WINDOWS:
                ensure_st(0, a_st)
                for st in b_sts:
                    ensure_st(1, st)
                for st in c_sts:
                    ensure_st(2, st)
                for rt in range(NRT):
                    As = st_groups(0, a_st)
                    Bs = [g for st in b_sts for g in st_groups(1, st)]
                    Cs = [g for st in c_sts for g in st_groups(2, st)]
                    for tile_groups in build_tiles(As, Bs, Cs):
                        emit_tile(tile_groups, rt, zcols[rt])
                        zcols[rt] += 1
                    if blk < 3 * NRT:
                        emit_gather_dot(blk)
                    blk += 1

            # ---------------- final reduction ----------------
            for rt in range(NRT):
                nc.vector.tensor_reduce(
                    out=zred[:, rt:rt + 1], in_=zbig[:, rt * 32:(rt + 1) * 32],
                    axis=mybir.AxisListType.X, op=ALU.add)
            nc.scalar.activation(logz[:], zred[:], ACTF.Ln)
            # loss8 = logz - (tl0 + ge1*(tl1-tl0) + ge2*(tl2-tl1))
            nc.vector.tensor_tensor(out=d1[:], in0=tl[1][:], in1=tl[0][:],
                                    op=ALU.subtract)
            nc.vector.tensor_tensor(out=d2[:], in0=tl[2][:], in1=tl[1][:],
                                    op=ALU.subtract)
            nc.vector.tensor_tensor(out=d1[:], in0=d1[:], in1=ge1[:],
                                    op=ALU.mult)
            nc.vector.tensor_tensor(out=d2[:], in0=d2[:], in1=ge2[:],
                                    op=ALU.mult)
            nc.vector.tensor_tensor(out=loss8[:], in0=logz[:], in1=tl[0][:],
                                    op=ALU.subtract)
            nc.vector.tensor_tensor(out=loss8[:], in0=loss8[:], in1=d1[:],
                                    op=ALU.subtract)
            nc.vector.tensor_tensor(out=loss8[:], in0=loss8[:], in1=d2[:],
                                    op=ALU.subtract)
            nc.vector.tensor_reduce(out=lossv[:], in_=loss8[:],
                                    axis=mybir.AxisListType.X, op=ALU.add)
            ps = psum.tile([P, ST], F32, tag="ps")
            nc.tensor.matmul(out=ps[0:1, 0:1], lhsT=lossv[:], rhs=ones[:],
                             start=True, stop=True)
            nc.scalar.mul(part[0:1, 0:1], ps[0:1, 0:1], 1.0 / float(B_T))
            nc.sync.dma_start(out=out_ext[:, :], in_=part[:])

    nc.compile()
    return nc


def _get_nc():
    global _NC_CACHE
    if _NC_CACHE is None:
        _NC_CACHE = _build_graph()
    return _NC_CACHE


def _make_in_maps(h, targets, W_head0, W_proj1, W_head1, W_proj2, W_head2):
    h = np.ascontiguousarray(np.asarray(h, dtype=np.float32)).reshape(B_T, D)
    t = np.asarray(targets).reshape(-1).astype(np.float32)
    w0 = np.ascontiguousarray(np.asarray(W_head0, dtype=np.float32))
    w1 = np.ascontiguousarray(np.asarray(W_head1, dtype=np.float32))
    w2 = np.ascontiguousarray(np.asarray(W_head2, dtype=np.float32))
    wp1 = np.ascontiguousarray(np.asarray(W_proj1, dtype=np.float32))
    wp2 = np.ascontiguousarray(np.asarray(W_proj2, dtype=np.float32))
    wt0 = np.ascontiguousarray(w0.T)
    wt1 = np.ascontiguousarray(w1.T)
    wt2 = np.ascontiguousarray(w2.T)

    in_maps = []
    for c in range(N_CORES):
        hc = h[c * RPC:(c + 1) * RPC]
        tc_ = t[c * RPC:(c + 1) * RPC]
        in_maps.append({
            "ht": np.ascontiguousarray(hc.T),
            "hr": hc,
            "tf": np.ascontiguousarray(tc_.reshape(NRT, P).T),
            "wp1": wp1, "wp2": wp2,
            "w0": w0, "w1": w1, "w2": w2,
            "wt0": wt0, "wt1": wt1, "wt2": wt2,
        })
    return in_maps


def kernel(h, targets, token_to_tier, token_to_idx,
           W_head0, W_proj1, W_head1, W_proj2, W_head2):
    in_maps = _make_in_maps(h, targets, W_head0, W_proj1, W_head1,
                            W_proj2, W_head2)
    nc = _get_nc()
    res = run_bass_kernel_spmd(nc, in_maps, core_ids=list(range(N_CORES)))
    total = sum(float(res.results[c]["out"][0, 0]) for c in range(N_CORES))
    return np.float32(total)


# revision 10
# speedup vs baseline: 1.2653x; 1.2653x over previous
"""Adaptive LM head (3-tier chunked softmax cross-entropy) on 8 TRN2 NeuronCores.

Strategy: data-parallel over B_T = 8192 rows (1024 rows/core; weights
replicated). Per core:
  - logits for each tier computed as bf16 matmuls (hT stationary, W streaming,
    f32->bf16 cast done in-flight by SWDGE DMA), PSUM accumulation over the
    contraction dim, 2048-wide vocab super-tiles.
  - ScalarE activation(Exp, accum_out=...) fuses exp + per-row sum in a single
    pass over each [128, 2048] logit tile (accum_out -> per-row partial Z).
  - tier units are interleaved (PE-heavy tier0 against ACT-heavy tier2) so
    TensorE stays dense/warm while ScalarE drains exp sums.
  - target logit = dot(feature_row, W[:, target]) computed exactly in f32:
    indirect-DMA gather of transposed-weight rows + fused scalar_tensor_tensor
    multiply-reduce, spread across the main loop.
  - per-core partial loss (sum_rows(log Z - target_logit)/8192) is the output;
    the host sums the 8 partials (the unshard step for a DP loss).
"""

import numpy as np

from concourse import bacc, bass, mybir
from concourse.bass import IndirectOffsetOnAxis
from concourse.bass_utils import run_bass_kernel_spmd
from concourse.tile import TileContext

F32 = mybir.dt.float32
BF16 = mybir.dt.bfloat16
I32 = mybir.dt.int32
FP8 = mybir.dt.float8e4
DR = mybir.MatmulPerfMode.DoubleRow
ALU = mybir.AluOpType
ACTF = mybir.ActivationFunctionType

P = 128
D = 1024
N_CORES = 8
RPC = 1024          # rows per core
NRT = RPC // P      # row tiles per core = 8
ST = 2048           # vocab super-tile width
NB = 512            # matmul free-dim tile (one PSUM bank)
NBM = 512           # main-loop moving-operand tile
V0, V1, V2 = 8192, 16384, 25681
PD1, PD2 = 256, 128
B_T = 8192

# Composite-tile schedule: each [128, 2048] PSUM tile packs four 512-col
# groups from DIFFERENT tiers (tier0 K=8 fill 1.7us, tier1 K=2 0.43us,
# tier2 K=1 0.21us) so every tile's matmul fill time roughly matches the
# ~2.0us ScalarE exp+sum drain -- the depth-2 PSUM ping-pong then keeps both
# engines dense. exp+accum sums all packed columns regardless of tier.
# windows: (tier0 st, [tier1 sts], [tier2 sts])
WINDOWS = [
    (0, [0, 1], [0, 1, 2]),
    (1, [2, 3], [3, 4, 5]),
    (2, [4, 5], [6, 7, 8]),
    (3, [6, 7], [9, 10, 11, 12]),
]

_NC_CACHE = None


def _ceil_div(a, b):
    return (a + b - 1) // b


def _build_graph():
    nc = bacc.Bacc("TRN2", target_bir_lowering=False, debug=False,
                   num_devices=N_CORES)

    ht_ext = nc.declare_dram_parameter("ht", [D, RPC], F32, isOutput=False)
    hr_ext = nc.declare_dram_parameter("hr", [RPC, D], F32, isOutput=False)
    tf_ext = nc.declare_dram_parameter("tf", [P, NRT], F32, isOutput=False)
    wp1_ext = nc.declare_dram_parameter("wp1", [D, PD1], F32, isOutput=False)
    wp2_ext = nc.declare_dram_parameter("wp2", [D, PD2], F32, isOutput=False)
    w0_ext = nc.declare_dram_parameter("w0", [D, V0], F32, isOutput=False)
    w1_ext = nc.declare_dram_parameter("w1", [PD1, V1], F32, isOutput=False)
    w2_ext = nc.declare_dram_parameter("w2", [PD2, V2], F32, isOutput=False)
    wt0_ext = nc.declare_dram_parameter("wt0", [V0, D], F32, isOutput=False)
    wt1_ext = nc.declare_dram_parameter("wt1", [V1, PD1], F32, isOutput=False)
    wt2_ext = nc.declare_dram_parameter("wt2", [V2, PD2], F32, isOutput=False)
    out_ext = nc.declare_dram_parameter("out", [1, 1], F32, isOutput=True)

    with TileContext(nc) as tc:
        with (
            tc.tile_pool(name="res", bufs=1) as res,
            tc.tile_pool(name="w0pool", bufs=2) as w0pool,
            tc.tile_pool(name="w1pool", bufs=4) as w1pool,
            tc.tile_pool(name="w2pool", bufs=6) as w2pool,
            tc.tile_pool(name="hrpool", bufs=2) as hrpool,
            tc.tile_pool(name="expool", bufs=3) as expool,
            tc.tile_pool(name="gpool", bufs=1) as gpool,
            tc.tile_pool(name="prodpool", bufs=1) as prodpool,
            tc.tile_pool(name="psum", bufs=2, space="PSUM") as psum,
        ):
            # ---------------- resident tiles ----------------
            ht_sb = res.tile([P, 8 * RPC], BF16, tag="ht")       # 8 d-chunks
            ht8_sb = res.tile([P, 8 * RPC], FP8, tag="ht8")      # fp8 copy for DR
            wp1_sb = res.tile([P, 8 * PD1], BF16, tag="wp1")
            wp2_sb = res.tile([P, 8 * PD2], BF16, tag="wp2")
            hp1T_sb = res.tile([P, 2 * RPC], FP8, tag="hp1T")
            hp2T_sb = res.tile([P, 1 * RPC], BF16, tag="hp2T")
            hp1r_sb = res.tile([P, NRT * PD1], F32, tag="hp1r")
            hp2r_sb = res.tile([P, NRT * PD2], F32, tag="hp2r")
            tf_sb = res.tile([P, NRT], F32, tag="tf")
            ge1 = res.tile([P, NRT], F32, tag="ge1")
            ge2 = res.tile([P, NRT], F32, tag="ge2")
            idxf = [res.tile([P, NRT], F32, tag=f"idxf{t}", name=f"idxf{t}")
                    for t in range(3)]
            idxi = [res.tile([P, NRT], I32, tag=f"idxi{t}", name=f"idxi{t}")
                    for t in range(3)]
            tl = [res.tile([P, NRT], F32, tag=f"tl{t}", name=f"tl{t}")
                  for t in range(3)]
            zbig = res.tile([P, NRT * 32], F32, tag="zbig")
            zred = res.tile([P, NRT], F32, tag="zred")
            logz = res.tile([P, NRT], F32, tag="logz")
            d1 = res.tile([P, NRT], F32, tag="d1")
            d2 = res.tile([P, NRT], F32, tag="d2")
            loss8 = res.tile([P, NRT], F32, tag="loss8")
            lossv = res.tile([P, 1], F32, tag="lossv")
            ones = res.tile([P, 1], F32, tag="ones")
            part = res.tile([1, 1], F32, tag="part")

            # ---------------- input staging ----------------
            # order matters for scheduling priority: ht (feeds everything),
            # then the first units' W slices arrive via the per-tier pools
            for k in range(8):
                nc.gpsimd.dma_start(
                    out=ht_sb[:, k * RPC:(k + 1) * RPC],
                    in_=ht_ext[k * P:(k + 1) * P, :])
                nc.gpsimd.dma_start(
                    out=ht8_sb[:, k * RPC:(k + 1) * RPC],
                    in_=ht_ext[k * P:(k + 1) * P, :])
            nc.sync.dma_start(out=tf_sb[:], in_=tf_ext[:, :])
            for k in range(8):
                nc.gpsimd.dma_start(
                    out=wp1_sb[:, k * PD1:(k + 1) * PD1],
                    in_=wp1_ext[k * P:(k + 1) * P, :])
                nc.gpsimd.dma_start(
                    out=wp2_sb[:, k * PD2:(k + 1) * PD2],
                    in_=wp2_ext[k * P:(k + 1) * P, :])

            nc.vector.memset(zbig[:], 0.0)
            nc.vector.memset(ones[:], 1.0)

            # ---------------- masks and in-tier indices ----------------
            nc.vector.tensor_scalar(out=ge1[:], in0=tf_sb[:], scalar1=float(V0),
                                    scalar2=None, op0=ALU.is_ge)
            nc.vector.tensor_scalar(out=ge2[:], in0=tf_sb[:],
                                    scalar1=float(V0 + V1), scalar2=None,
                                    op0=ALU.is_ge)
            nc.vector.tensor_scalar(out=idxf[0][:], in0=tf_sb[:],
                                    scalar1=float(V0 - 1), scalar2=None,
                                    op0=ALU.min)
            nc.vector.tensor_scalar(out=idxf[1][:], in0=tf_sb[:],
                                    scalar1=-float(V0), scalar2=0.0,
                                    op0=ALU.add, op1=ALU.max)
            nc.vector.tensor_scalar(out=idxf[1][:], in0=idxf[1][:],
                                    scalar1=float(V1 - 1), scalar2=None,
                                    op0=ALU.min)
            nc.vector.tensor_scalar(out=idxf[2][:], in0=tf_sb[:],
                                    scalar1=-float(V0 + V1), scalar2=0.0,
                                    op0=ALU.add, op1=ALU.max)
            nc.vector.tensor_scalar(out=idxf[2][:], in0=idxf[2][:],
                                    scalar1=float(V2 - 1), scalar2=None,
                                    op0=ALU.min)
            for t in range(3):
                nc.vector.tensor_copy(out=idxi[t][:], in_=idxf[t][:])

            # ---------------- projections ----------------
            for m in range(PD1 // P):
                for rb in range(RPC // NB):
                    ps = psum.tile([P, ST], F32, tag="ps")
                    for k in range(8):
                        nc.tensor.matmul(
                            out=ps[:, :NB],
                            lhsT=wp1_sb[:, k * PD1 + m * P: k * PD1 + (m + 1) * P],
                            rhs=ht_sb[:, k * RPC + rb * NB: k * RPC + (rb + 1) * NB],
                            start=(k == 0), stop=(k == 7))
                    nc.vector.tensor_copy(
                        out=hp1T_sb[:, m * RPC + rb * NB: m * RPC + (rb + 1) * NB],
                        in_=ps[:, :NB])
            for rb in range(RPC // NB):
                ps = psum.tile([P, ST], F32, tag="ps")
                for k in range(8):
                    nc.tensor.matmul(
                        out=ps[:, :NB],
                        lhsT=wp2_sb[:, k * PD2:(k + 1) * PD2],
                        rhs=ht_sb[:, k * RPC + rb * NB: k * RPC + (rb + 1) * NB],
                        start=(k == 0), stop=(k == 7))
                nc.vector.tensor_copy(
                    out=hp2T_sb[:, rb * NB:(rb + 1) * NB], in_=ps[:, :NB])

            for rt in range(NRT):
                ps = psum.tile([P, ST], F32, tag="ps")
                for k in range(8):
                    nc.tensor.matmul(
                        out=ps[:, :PD1],
                        lhsT=ht_sb[:, k * RPC + rt * P: k * RPC + rt * P + P],
                        rhs=wp1_sb[:, k * PD1:(k + 1) * PD1],
                        start=(k == 0), stop=(k == 7))
                nc.vector.tensor_copy(
                    out=hp1r_sb[:, rt * PD1:(rt + 1) * PD1], in_=ps[:, :PD1])
            for rt in range(NRT):
                ps = psum.tile([P, ST], F32, tag="ps")
                for k in range(8):
                    nc.tensor.matmul(
                        out=ps[:, :PD2],
                        lhsT=ht_sb[:, k * RPC + rt * P: k * RPC + rt * P + P],
                        rhs=wp2_sb[:, k * PD2:(k + 1) * PD2],
                        start=(k == 0), stop=(k == 7))
                nc.vector.tensor_copy(
                    out=hp2r_sb[:, rt * PD2:(rt + 1) * PD2], in_=ps[:, :PD2])

            # ---------------- interleaved main units ----------------
            # (V, K, w_ext, lhsT_sb, wpool, nchunks, wdtype, doublerow)
            tiers = {
                0: (V0, 8, w0_ext, ht8_sb, w0pool, 8, FP8, True),
                1: (V1, 2, w1_ext, hp1T_sb, w1pool, 2, FP8, True),
                2: (V2, 1, w2_ext, hp2T_sb, w2pool, 1, BF16, False),
            }
            gather_src = [wt0_ext, wt1_ext, wt2_ext]
            gdim = [D, PD1, PD2]
            gmax = [V0 - 1, V1 - 1, V2 - 1]

            def emit_gather_dot(i):
                rt, t = divmod(i, 3)
                if t == 0:
                    hr_t = hrpool.tile([P, D], F32, tag="hrt", name="hrt")
                    nc.sync.dma_start(out=hr_t[:],
                                      in_=hr_ext[rt * P:(rt + 1) * P, :])
                    feat_ap = hr_t[:]
                elif t == 1:
                    feat_ap = hp1r_sb[:, rt * PD1:(rt + 1) * PD1]
                else:
                    feat_ap = hp2r_sb[:, rt * PD2:(rt + 1) * PD2]
                g = gpool.tile([P, gdim[t]], F32, tag=f"g{t}", name=f"g{t}")
                nc.gpsimd.indirect_dma_start(
                    out=g[:], out_offset=None,
                    in_=gather_src[t][:, :],
                    in_offset=IndirectOffsetOnAxis(
                        ap=idxi[t][:, rt:rt + 1], axis=0),
                    bounds_check=gmax[t], oob_is_err=False)
                prod = prodpool.tile([P, D], F32, tag="prod")
                nc.vector.scalar_tensor_tensor(
                    out=prod[:, :gdim[t]],
                    in0=feat_ap, scalar=1.0, in1=g[:],
                    op0=ALU.mult, op1=ALU.mult,
                    accum_out=tl[t][:, rt:rt + 1])

            st_wtile = {}

            def ensure_st(tier, st):
                if (tier, st) in st_wtile:
                    return
                V, K, w_ext, lhsT_sb, wpool, nchunks, wdt, dr = tiers[tier]
                w = min(ST, V - st * ST)
                wtile = wpool.tile([P, nchunks * ST], wdt,
                                   tag=f"w{tier}", name=f"w{tier}")
                for k in range(K):
                    nc.gpsimd.dma_start(
                        out=wtile[:, k * ST: k * ST + w],
                        in_=w_ext[k * P:(k + 1) * P, st * ST: st * ST + w])
                st_wtile[(tier, st)] = wtile

            def emit_tile(groups, rt, zcol):
                # groups: list of (tier, st, g, gw); packed into one psum tile
                ps = psum.tile([P, ST], F32, tag="ps")
                off = 0
                for (tier, st, g, gw) in groups:
                    V, K, w_ext, lhsT_sb, wpool, nchunks, wdt, dr = tiers[tier]
                    wtile = st_wtile[(tier, st)]
                    if dr:
                        lv = lhsT_sb[:].rearrange("p (k r) -> p k r", k=nchunks)
                        wv = wtile[:].rearrange("p (k c) -> p k c", k=nchunks)
                        for pr in range(K // 2):
                            nc.tensor.matmul(
                                out=ps[:, off: off + gw],
                                lhsT=lv[:, 2 * pr: 2 * pr + 2,
                                        rt * P: rt * P + P],
                                rhs=wv[:, 2 * pr: 2 * pr + 2,
                                       g * NB: g * NB + gw],
                                start=(pr == 0), stop=(pr == K // 2 - 1),
                                perf_mode=DR)
                    else:
                        for k in range(K):
                            nc.tensor.matmul(
                                out=ps[:, off: off + gw],
                                lhsT=lhsT_sb[:, k * RPC + rt * P:
                                             k * RPC + rt * P + P],
                                rhs=wtile[:, k * ST + g * NB:
                                          k * ST + g * NB + gw],
                                start=(k == 0), stop=(k == K - 1))
                    off += gw
                ex = expool.tile([P, ST], BF16, tag="ex")
                nc.scalar.activation(
                    ex[:, :off], ps[:, :off], ACTF.Exp,
                    accum_out=zbig[:, rt * 32 + zcol: rt * 32 + zcol + 1])

            def build_tiles(As, Bs, Cs):
                tiles = []
                ia = ib = ic = 0
                while ia < len(As):
                    t = [As[ia]]; ia += 1
                    if ib < len(Bs):
                        t.append(Bs[ib]); ib += 1
                    t += Cs[ic:ic + 2]; ic += 2
                    tiles.append(t)
                while ib < len(Bs):
                    t = Bs[ib:ib + 2]; ib += 2
                    t += Cs[ic:ic + 2]; ic += 2
                    tiles.append(t)
                while ic < len(Cs):
                    t = Cs[ic:ic + 4]; ic += len(Cs[ic:ic + 4])
                    tiles.append(t)
                return tiles

            def st_groups(tier, st):
                V = tiers[tier][0]
                w = min(ST, V - st * ST)
                gs = []
                for g in range(_ceil_div(w, NB)):
                    gs.append((tier, st, g, min(NB, w - g * NB)))
                return gs

            zcols = [0] * NRT
            blk = 0
            for (a_st, b_sts, c_sts) in WINDOWS:
                ensure_st(0, a_st)
                for st in b_sts:
                    ensure_st(1, st)
                for st in c_sts:
                    ensure_st(2, st)
                for rt in range(NRT):
                    As = st_groups(0, a_st)
                    Bs = [g for st in b_sts for g in st_groups(1, st)]
                    Cs = [g for st in c_sts for g in st_groups(2, st)]
                    for tile_groups in build_tiles(As, Bs, Cs):
                        emit_tile(tile_groups, rt, zcols[rt])
                        zcols[rt] += 1
                    if blk < 3 * NRT:
                        emit_gather_dot(blk)
                    blk += 1

            # ---------------- final reduction ----------------
            for rt in range(NRT):
                nc.vector.tensor_reduce(
                    out=zred[:, rt:rt + 1], in_=zbig[:, rt * 32:(rt + 1) * 32],
                    axis=mybir.AxisListType.X, op=ALU.add)
            nc.scalar.activation(logz[:], zred[:], ACTF.Ln)
            # loss8 = logz - (tl0 + ge1*(tl1-tl0) + ge2*(tl2-tl1))
            nc.vector.tensor_tensor(out=d1[:], in0=tl[1][:], in1=tl[0][:],
                                    op=ALU.subtract)
            nc.vector.tensor_tensor(out=d2[:], in0=tl[2][:], in1=tl[1][:],
                                    op=ALU.subtract)
            nc.vector.tensor_tensor(out=d1[:], in0=d1[:], in1=ge1[:],
                                    op=ALU.mult)
            nc.vector.tensor_tensor(out=d2[:], in0=d2[:], in1=ge2[:],
                                    op=ALU.mult)
            nc.vector.tensor_tensor(out=loss8[:], in0=logz[:], in1=tl[0][:],
                                    op=ALU.subtract)
            nc.vector.tensor_tensor(out=loss8[:], in0=loss8[:], in1=d1[:],
                                    op=ALU.subtract)
            nc.vector.tensor_tensor(out=loss8[:], in0=loss8[:], in1=d2[:],
                                    op=ALU.subtract)
            nc.vector.tensor_reduce(out=lossv[:], in_=loss8[:],
                                    axis=mybir.AxisListType.X, op=ALU.add)
            ps = psum.tile([P, ST], F32, tag="ps")
            nc.tensor.matmul(out=ps[0:1, 0:1], lhsT=lossv[:], rhs=ones[:],
                             start=True, stop=True)
            nc.scalar.mul(part[0:1, 0:1], ps[0:1, 0:1], 1.0 / float(B_T))
            nc.sync.dma_start(out=out_ext[:, :], in_=part[:])

    nc.compile()
    return nc


def _get_nc():
    global _NC_CACHE
    if _NC_CACHE is None:
        _NC_CACHE = _build_graph()
    return _NC_CACHE


def _make_in_maps(h, targets, W_head0, W_proj1, W_head1, W_proj2, W_head2):
    h = np.ascontiguousarray(np.asarray(h, dtype=np.float32)).reshape(B_T, D)
    t = np.asarray(targets).reshape(-1).astype(np.float32)
    w0 = np.ascontiguousarray(np.asarray(W_head0, dtype=np.float32))
    w1 = np.ascontiguousarray(np.asarray(W_head1, dtype=np.float32))
    w2 = np.ascontiguousarray(np.asarray(W_head2, dtype=np.float32))
    wp1 = np.ascontiguousarray(np.asarray(W_proj1, dtype=np.float32))
    wp2 = np.ascontiguousarray(np.asarray(W_proj2, dtype=np.float32))
    wt0 = np.ascontiguousarray(w0.T)
    wt1 = np.ascontiguousarray(w1.T)
    wt2 = np.ascontiguousarray(w2.T)

    in_maps = []
    for c in range(N_CORES):
        hc = h[c * RPC:(c + 1) * RPC]
        tc_ = t[c * RPC:(c + 1) * RPC]
        in_maps.append({
            "ht": np.ascontiguousarray(hc.T),
            "hr": hc,
            "tf": np.ascontiguousarray(tc_.reshape(NRT, P).T),
            "wp1": wp1, "wp2": wp2,
            "w0": w0, "w1": w1, "w2": w2,
            "wt0": wt0, "wt1": wt1, "wt2": wt2,
        })
    return in_maps


def kernel(h, targets, token_to_tier, token_to_idx,
           W_head0, W_proj1, W_head1, W_proj2, W_head2):
    in_maps = _make_in_maps(h, targets, W_head0, W_proj1, W_head1,
                            W_proj2, W_head2)
    nc = _get_nc()
    res = run_bass_kernel_spmd(nc, in_maps, core_ids=list(range(N_CORES)))
    total = sum(float(res.results[c]["out"][0, 0]) for c in range(N_CORES))
    return np.float32(total)


# revision 11
# speedup vs baseline: 1.3782x; 1.0893x over previous
"""Adaptive LM head (3-tier chunked softmax cross-entropy) on 8 TRN2 NeuronCores.

Strategy: data-parallel over B_T = 8192 rows (1024 rows/core; weights
replicated). Per core:
  - tier logits via fp8 DoubleRow matmuls (tiers 0/1) and fp8 matmuls
    (tier 2); weights stream from HBM as f32 and are cast in-flight by the
    SWDGE DMA engines. PSUM accumulation over the contraction dim.
  - ScalarE activation(Exp, accum_out=...) fuses exp + per-row sum in a single
    pass over each [128, 2048] logit tile; the schedule packs 512-col groups
    from different tiers into composite tiles and is ACT-bound throughout.
  - target logit = dot(feature_row, W[:, target]) computed in f32/bf16:
    indirect-DMA gather of transposed-weight rows + fused scalar_tensor_tensor
    multiply-reduce, spread through the main stream.
  - per-core partial loss (sum_rows(log Z - target_logit)/8192) is the output;
    the host sums the 8 partials (the unshard step for a DP loss).
"""

import numpy as np

from concourse import bacc, bass, mybir
from concourse.bass import IndirectOffsetOnAxis
from concourse.bass_utils import run_bass_kernel_spmd
from concourse.tile import TileContext

F32 = mybir.dt.float32
BF16 = mybir.dt.bfloat16
I32 = mybir.dt.int32
FP8 = mybir.dt.float8e4
DR = mybir.MatmulPerfMode.DoubleRow
ALU = mybir.AluOpType
ACTF = mybir.ActivationFunctionType

P = 128
D = 1024
N_CORES = 8
RPC = 1024          # rows per core
NRT = RPC // P      # row tiles per core = 8
ST = 2048           # vocab super-tile width
NB = 512            # 512-col group (one PSUM bank)
V0, V1, V2 = 8192, 16384, 25681
PD1, PD2 = 256, 128
B_T = 8192

# windows: (tier0 st, [tier1 sts], [tier2 sts]); within a window each psum
# tile packs groups from different tiers so fills stay balanced vs the
# ScalarE exp+sum drain.
WINDOWS = [
    (0, [0, 1], [0, 1, 2]),
    (1, [2, 3], [3, 4, 5]),
    (2, [4, 5], [6, 7, 8]),
    (3, [6, 7], [9, 10, 11, 12]),
]
GATHER_BLK0 = 8   # first schedule block that may emit a gather/dot

_NC_CACHE = None


def _ceil_div(a, b):
    return (a + b - 1) // b


def _build_graph():
    nc = bacc.Bacc("TRN2", target_bir_lowering=False, debug=False,
                   num_devices=N_CORES)

    ht_ext = nc.declare_dram_parameter("ht", [D, RPC], F32, isOutput=False)
    hr_ext = nc.declare_dram_parameter("hr", [RPC, D], F32, isOutput=False)
    tf_ext = nc.declare_dram_parameter("tf", [P, NRT], F32, isOutput=False)
    wp1_ext = nc.declare_dram_parameter("wp1", [D, PD1], F32, isOutput=False)
    wp2_ext = nc.declare_dram_parameter("wp2", [D, PD2], F32, isOutput=False)
    w0_ext = nc.declare_dram_parameter("w0", [D, V0], F32, isOutput=False)
    w1_ext = nc.declare_dram_parameter("w1", [PD1, V1], F32, isOutput=False)
    w2_ext = nc.declare_dram_parameter("w2", [PD2, V2], F32, isOutput=False)
    wt0_ext = nc.declare_dram_parameter("wt0", [V0, D], F32, isOutput=False)
    wt1_ext = nc.declare_dram_parameter("wt1", [V1, PD1], F32, isOutput=False)
    wt2_ext = nc.declare_dram_parameter("wt2", [V2, PD2], F32, isOutput=False)
    out_ext = nc.declare_dram_parameter("out", [1, 1], F32, isOutput=True)

    with TileContext(nc) as tc:
        with (
            tc.tile_pool(name="res", bufs=1) as res,
            tc.tile_pool(name="w0pool", bufs=2) as w0pool,
            tc.tile_pool(name="w1pool", bufs=4) as w1pool,
            tc.tile_pool(name="w2pool", bufs=6) as w2pool,
            tc.tile_pool(name="hrpool", bufs=2) as hrpool,
            tc.tile_pool(name="expool", bufs=3) as expool,
            tc.tile_pool(name="gpool", bufs=1) as gpool,
            tc.tile_pool(name="prodpool", bufs=1) as prodpool,
            tc.tile_pool(name="psum", bufs=2, space="PSUM") as psum,
        ):
            # ---------------- resident tiles ----------------
            ht8_sb = res.tile([P, 8 * RPC], FP8, tag="ht8")
            wp1_8 = res.tile([P, 8 * PD1], FP8, tag="wp18")
            wp2_8 = res.tile([P, 8 * PD2], FP8, tag="wp28")
            ht_sb = res.tile([P, 8 * RPC], BF16, tag="ht")
            wp1_sb = res.tile([P, 8 * PD1], BF16, tag="wp1")
            wp2_sb = res.tile([P, 8 * PD2], BF16, tag="wp2")
            hp1T_sb = res.tile([P, 2 * RPC], FP8, tag="hp1T")
            hp2T_sb = res.tile([P, 1 * RPC], FP8, tag="hp2T")
            hp1r_sb = res.tile([P, NRT * PD1], F32, tag="hp1r")
            hp2r_sb = res.tile([P, NRT * PD2], F32, tag="hp2r")
            tf_sb = res.tile([P, NRT], F32, tag="tf")
            ge1 = res.tile([P, NRT], F32, tag="ge1")
            ge2 = res.tile([P, NRT], F32, tag="ge2")
            idxf = [res.tile([P, NRT], F32, tag=f"idxf{t}", name=f"idxf{t}")
                    for t in range(3)]
            idxi = [res.tile([P, NRT], I32, tag=f"idxi{t}", name=f"idxi{t}")
                    for t in range(3)]
            tl = [res.tile([P, NRT], F32, tag=f"tl{t}", name=f"tl{t}")
                  for t in range(3)]
            zbig = res.tile([P, NRT * 32], F32, tag="zbig")
            zred = res.tile([P, NRT], F32, tag="zred")
            logz = res.tile([P, NRT], F32, tag="logz")
            d1 = res.tile([P, NRT], F32, tag="d1")
            d2 = res.tile([P, NRT], F32, tag="d2")
            loss8 = res.tile([P, NRT], F32, tag="loss8")
            lossv = res.tile([P, 1], F32, tag="lossv")
            ones = res.tile([P, 1], F32, tag="ones")
            part = res.tile([1, 1], F32, tag="part")

            def load_chunked(dst, src, width):
                nc.gpsimd.dma_start(
                    out=dst[:].rearrange("p (k c) -> p k c", k=8),
                    in_=src[:, :].rearrange("(k p) c -> p k c", p=P))

            # fp8 staging first: these gate the projections and tier0
            load_chunked(ht8_sb, ht_ext, RPC)
            load_chunked(wp1_8, wp1_ext, PD1)
            load_chunked(wp2_8, wp2_ext, PD2)
            nc.sync.dma_start(out=tf_sb[:], in_=tf_ext[:, :])

            nc.vector.memset(zbig[:], 0.0)
            nc.vector.memset(ones[:], 1.0)

            # ---------------- masks and in-tier indices ----------------
            nc.vector.tensor_scalar(out=ge1[:], in0=tf_sb[:], scalar1=float(V0),
                                    scalar2=None, op0=ALU.is_ge)
            nc.vector.tensor_scalar(out=ge2[:], in0=tf_sb[:],
                                    scalar1=float(V0 + V1), scalar2=None,
                                    op0=ALU.is_ge)
            nc.vector.tensor_scalar(out=idxf[0][:], in0=tf_sb[:],
                                    scalar1=float(V0 - 1), scalar2=None,
                                    op0=ALU.min)
            nc.vector.tensor_scalar(out=idxf[1][:], in0=tf_sb[:],
                                    scalar1=-float(V0), scalar2=0.0,
                                    op0=ALU.add, op1=ALU.max)
            nc.vector.tensor_scalar(out=idxf[1][:], in0=idxf[1][:],
                                    scalar1=float(V1 - 1), scalar2=None,
                                    op0=ALU.min)
            nc.vector.tensor_scalar(out=idxf[2][:], in0=tf_sb[:],
                                    scalar1=-float(V0 + V1), scalar2=0.0,
                                    op0=ALU.add, op1=ALU.max)
            nc.vector.tensor_scalar(out=idxf[2][:], in0=idxf[2][:],
                                    scalar1=float(V2 - 1), scalar2=None,
                                    op0=ALU.min)
            for t in range(3):
                nc.vector.tensor_copy(out=idxi[t][:], in_=idxf[t][:])

            # ---------------- fp8 DoubleRow projections (hp1T, hp2T) -------
            ht8v = ht8_sb[:].rearrange("p (k r) -> p k r", k=8)
            wp18v = wp1_8[:].rearrange("p (k c) -> p k c", k=8)
            wp28v = wp2_8[:].rearrange("p (k c) -> p k c", k=8)
            for m in range(PD1 // P):
                for rb in range(RPC // NB):
                    ps = psum.tile([P, ST], F32, tag="ps")
                    for pr in range(4):
                        nc.tensor.matmul(
                            out=ps[:, :NB],
                            lhsT=wp18v[:, 2 * pr: 2 * pr + 2,
                                       m * P:(m + 1) * P],
                            rhs=ht8v[:, 2 * pr: 2 * pr + 2,
                                     rb * NB:(rb + 1) * NB],
                            start=(pr == 0), stop=(pr == 3), perf_mode=DR)
                    nc.vector.tensor_copy(
                        out=hp1T_sb[:, m * RPC + rb * NB:
                                    m * RPC + (rb + 1) * NB],
                        in_=ps[:, :NB])
            for rb in range(RPC // NB):
                ps = psum.tile([P, ST], F32, tag="ps")
                for pr in range(4):
                    nc.tensor.matmul(
                        out=ps[:, :NB],
                        lhsT=wp28v[:, 2 * pr: 2 * pr + 2, 0:P],
                        rhs=ht8v[:, 2 * pr: 2 * pr + 2,
                                 rb * NB:(rb + 1) * NB],
                        start=(pr == 0), stop=(pr == 3), perf_mode=DR)
                nc.vector.tensor_copy(
                    out=hp2T_sb[:, rb * NB:(rb + 1) * NB], in_=ps[:, :NB])

            # ---------------- main stream ----------------
            # (V, K, w_ext, lhsT_sb, wpool, nchunks, wdtype, doublerow)
            tiers = {
                0: (V0, 8, w0_ext, ht8_sb, w0pool, 8, FP8, True),
                1: (V1, 2, w1_ext, hp1T_sb, w1pool, 2, FP8, True),
                2: (V2, 1, w2_ext, hp2T_sb, w2pool, 1, FP8, False),
            }
            gather_src = [wt0_ext, wt1_ext, wt2_ext]
            gdim = [D, PD1, PD2]
            gmax = [V0 - 1, V1 - 1, V2 - 1]
            st_wtile = {}

            def ensure_st(tier, st):
                if (tier, st) in st_wtile:
                    return
                V, K, w_ext, lhsT_sb, wpool, nchunks, wdt, dr = tiers[tier]
                w = min(ST, V - st * ST)
                wtile = wpool.tile([P, nchunks * ST], wdt,
                                   tag=f"w{tier}", name=f"w{tier}")
                for k in range(K):
                    nc.gpsimd.dma_start(
                        out=wtile[:, k * ST: k * ST + w],
                        in_=w_ext[k * P:(k + 1) * P, st * ST: st * ST + w])
                st_wtile[(tier, st)] = wtile

            def emit_rows_proj(rt, t):
                # bf16 rows-orientation projection for the target dot
                pd = PD1 if t == 1 else PD2
                wsrc = wp1_sb if t == 1 else wp2_sb
                dst = hp1r_sb if t == 1 else hp2r_sb
                ps = psum.tile([P, ST], F32, tag="ps")
                for k in range(8):
                    nc.tensor.matmul(
                        out=ps[:, :pd],
                        lhsT=ht_sb[:, k * RPC + rt * P: k * RPC + rt * P + P],
                        rhs=wsrc[:, k * pd:(k + 1) * pd],
                        start=(k == 0), stop=(k == 7))
                nc.vector.tensor_copy(
                    out=dst[:, rt * pd:(rt + 1) * pd], in_=ps[:, :pd])

            def emit_gather_dot(i):
                rt, t = divmod(i, 3)
                if t == 0:
                    hr_t = hrpool.tile([P, D], F32, tag="hrt", name="hrt")
                    nc.sync.dma_start(out=hr_t[:],
                                      in_=hr_ext[rt * P:(rt + 1) * P, :])
                    feat_ap = hr_t[:]
                elif t == 1:
                    emit_rows_proj(rt, 1)
                    feat_ap = hp1r_sb[:, rt * PD1:(rt + 1) * PD1]
                else:
                    emit_rows_proj(rt, 2)
                    feat_ap = hp2r_sb[:, rt * PD2:(rt + 1) * PD2]
                g = gpool.tile([P, gdim[t]], F32, tag=f"g{t}", name=f"g{t}")
                nc.gpsimd.indirect_dma_start(
                    out=g[:], out_offset=None,
                    in_=gather_src[t][:, :],
                    in_offset=IndirectOffsetOnAxis(
                        ap=idxi[t][:, rt:rt + 1], axis=0),
                    bounds_check=gmax[t], oob_is_err=False)
                prod = prodpool.tile([P, D], F32, tag="prod")
                nc.vector.scalar_tensor_tensor(
                    out=prod[:, :gdim[t]],
                    in0=feat_ap, scalar=1.0, in1=g[:],
                    op0=ALU.mult, op1=ALU.mult,
                    accum_out=tl[t][:, rt:rt + 1])

            def emit_tile(groups, rt, zcol):
                ps = psum.tile([P, ST], F32, tag="ps")
                off = 0
                for (tier, st, g, gw) in groups:
                    V, K, w_ext, lhsT_sb, wpool, nchunks, wdt, dr = tiers[tier]
                    wtile = st_wtile[(tier, st)]
                    if dr:
                        lv = lhsT_sb[:].rearrange("p (k r) -> p k r",
                                                  k=nchunks)
                        wv = wtile[:].rearrange("p (k c) -> p k c", k=nchunks)
                        for pr in range(K // 2):
                            nc.tensor.matmul(
                                out=ps[:, off: off + gw],
                                lhsT=lv[:, 2 * pr: 2 * pr + 2,
                                        rt * P: rt * P + P],
                                rhs=wv[:, 2 * pr: 2 * pr + 2,
                                       g * NB: g * NB + gw],
                                start=(pr == 0), stop=(pr == K // 2 - 1),
                                perf_mode=DR)
                    else:
                        for k in range(K):
                            nc.tensor.matmul(
                                out=ps[:, off: off + gw],
                                lhsT=lhsT_sb[:, k * RPC + rt * P:
                                             k * RPC + rt * P + P],
                                rhs=wtile[:, k * ST + g * NB:
                                          k * ST + g * NB + gw],
                                start=(k == 0), stop=(k == K - 1))
                    off += gw
                ex = expool.tile([P, ST], BF16, tag="ex")
                nc.scalar.activation(
                    ex[:, :off], ps[:, :off], ACTF.Exp,
                    accum_out=zbig[:, rt * 32 + zcol: rt * 32 + zcol + 1])

            def st_groups(tier, st):
                V = tiers[tier][0]
                w = min(ST, V - st * ST)
                return [(tier, st, g, min(NB, w - g * NB))
                        for g in range(_ceil_div(w, NB))]

            def build_tiles(As, Bs, Cs):
                # light (B/C-only) tiles first, then the A-bearing tiles
                tiles = []
                na = len(As)
                n_light_b = max(0, len(Bs) - na)
                lb = 0
                ic = 0
                while lb + 2 <= n_light_b:
                    tiles.append(Bs[lb:lb + 2] + Cs[ic:ic + 2])
                    lb += 2; ic += 2
                ib = lb
                for ia in range(na):
                    t = [As[ia]]
                    if ib < len(Bs):
                        t.append(Bs[ib]); ib += 1
                    t += Cs[ic:ic + 2]; ic += 2
                    tiles.append(t)
                while ic < len(Cs):
                    t = Cs[ic:ic + 4]; ic += len(Cs[ic:ic + 4])
                    tiles.append(t)
                return tiles

            zcols = [0] * NRT
            blk = 0
            gi = 0
            for wi, (a_st, b_sts, c_sts) in enumerate(WINDOWS):
                for st in c_sts:
                    ensure_st(2, st)
                for st in b_sts:
                    ensure_st(1, st)
                ensure_st(0, a_st)
                if wi == 0:
                    # bf16 staging for the deferred rows-projections; queued
                    # after window0's W slices so they don't delay the stream
                    load_chunked(ht_sb, ht_ext, RPC)
                    load_chunked(wp1_sb, wp1_ext, PD1)
                    load_chunked(wp2_sb, wp2_ext, PD2)
                for rt in range(NRT):
                    As = st_groups(0, a_st)
                    Bs = [g for st in b_sts for g in st_groups(1, st)]
                    Cs = [g for st in c_sts for g in st_groups(2, st)]
                    for tile_groups in build_tiles(As, Bs, Cs):
                        emit_tile(tile_groups, rt, zcols[rt])
                        zcols[rt] += 1
                    if blk >= GATHER_BLK0 and gi < 3 * NRT:
                        emit_gather_dot(gi)
                        gi += 1
                    blk += 1
            while gi < 3 * NRT:
                emit_gather_dot(gi)
                gi += 1

            # ---------------- final reduction ----------------
            for rt in range(NRT):
                nc.vector.tensor_reduce(
                    out=zred[:, rt:rt + 1], in_=zbig[:, rt * 32:(rt + 1) * 32],
                    axis=mybir.AxisListType.X, op=ALU.add)
            nc.scalar.activation(logz[:], zred[:], ACTF.Ln)
            # loss8 = logz - (tl0 + ge1*(tl1-tl0) + ge2*(tl2-tl1))
            nc.vector.tensor_tensor(out=d1[:], in0=tl[1][:], in1=tl[0][:],
                                    op=ALU.subtract)
            nc.vector.tensor_tensor(out=d2[:], in0=tl[2][:], in1=tl[1][:],
                                    op=ALU.subtract)
            nc.vector.tensor_tensor(out=d1[:], in0=d1[:], in1=ge1[:],
                                    op=ALU.mult)
            nc.vector.tensor_tensor(out=d2[:], in0=d2[:], in1=ge2[:],
                                    op=ALU.mult)
            nc.vector.tensor_tensor(out=loss8[:], in0=logz[:], in1=tl[0][:],
                                    op=ALU.subtract)
            nc.vector.tensor_tensor(out=loss8[:], in0=loss8[:], in1=d1[:],
                                    op=ALU.subtract)
            nc.vector.tensor_tensor(out=loss8[:], in0=loss8[:], in1=d2[:],
                                    op=ALU.subtract)
            nc.vector.tensor_reduce(out=lossv[:], in_=loss8[:],
                                    axis=mybir.AxisListType.X, op=ALU.add)
            ps = psum.tile([P, ST], F32, tag="ps")
            nc.tensor.matmul(out=ps[0:1, 0:1], lhsT=lossv[:], rhs=ones[:],
                             start=True, stop=True)
            nc.scalar.mul(part[0:1, 0:1], ps[0:1, 0:1], 1.0 / float(B_T))
            nc.sync.dma_start(out=out_ext[:, :], in_=part[:])

    nc.compile()
    return nc


def _get_nc():
    global _NC_CACHE
    if _NC_CACHE is None:
        _NC_CACHE = _build_graph()
    return _NC_CACHE


def _make_in_maps(h, targets, W_head0, W_proj1, W_head1, W_proj2, W_head2):
    h = np.ascontiguousarray(np.asarray(h, dtype=np.float32)).reshape(B_T, D)
    t = np.asarray(targets).reshape(-1).astype(np.float32)
    w0 = np.ascontiguousarray(np.asarray(W_head0, dtype=np.float32))
    w1 = np.ascontiguousarray(np.asarray(W_head1, dtype=np.float32))
    w2 = np.ascontiguousarray(np.asarray(W_head2, dtype=np.float32))
    wp1 = np.ascontiguousarray(np.asarray(W_proj1, dtype=np.float32))
    wp2 = np.ascontiguousarray(np.asarray(W_proj2, dtype=np.float32))
    wt0 = np.ascontiguousarray(w0.T)
    wt1 = np.ascontiguousarray(w1.T)
    wt2 = np.ascontiguousarray(w2.T)

    in_maps = []
    for c in range(N_CORES):
        hc = h[c * RPC:(c + 1) * RPC]
        tc_ = t[c * RPC:(c + 1) * RPC]
        in_maps.append({
            "ht": np.ascontiguousarray(hc.T),
            "hr": hc,
            "tf": np.ascontiguousarray(tc_.reshape(NRT, P).T),
            "wp1": wp1, "wp2": wp2,
            "w0": w0, "w1": w1, "w2": w2,
            "wt0": wt0, "wt1": wt1, "wt2": wt2,
        })
    return in_maps


def kernel(h, targets, token_to_tier, token_to_idx,
           W_head0, W_proj1, W_head1, W_proj2, W_head2):
    in_maps = _make_in_maps(h, targets, W_head0, W_proj1, W_head1,
                            W_proj2, W_head2)
    nc = _get_nc()
    res = run_bass_kernel_spmd(nc, in_maps, core_ids=list(range(N_CORES)))
    total = sum(float(res.results[c]["out"][0, 0]) for c in range(N_CORES))
    return np.float32(total)


# revision 12
# speedup vs baseline: 1.4438x; 1.0476x over previous
"""Adaptive LM head (3-tier chunked softmax cross-entropy) on 8 TRN2 NeuronCores.

Strategy: data-parallel over B_T = 8192 rows (1024 rows/core; weights
replicated). Per core:
  - tier logits via fp8 DoubleRow matmuls (tiers 0/1) and fp8 matmuls
    (tier 2); weights stream from HBM as f32 and are cast in-flight by the
    SWDGE DMA engines. PSUM accumulation over the contraction dim.
  - ScalarE activation(Exp, accum_out=...) fuses exp + per-row sum in a single
    pass over each [128, 2048] logit tile; the schedule packs 512-col groups
    from different tiers into composite tiles and is ACT-bound throughout.
  - target logit = dot(feature_row, W[:, target]) computed in f32/bf16:
    indirect-DMA gather of transposed-weight rows + fused scalar_tensor_tensor
    multiply-reduce, spread through the main stream.
  - per-core partial loss (sum_rows(log Z - target_logit)/8192) is the output;
    the host sums the 8 partials (the unshard step for a DP loss).
"""

import numpy as np

from concourse import bacc, bass, mybir
from concourse.bass import IndirectOffsetOnAxis
from concourse.bass_utils import run_bass_kernel_spmd
from concourse.tile import TileContext

F32 = mybir.dt.float32
BF16 = mybir.dt.bfloat16
I32 = mybir.dt.int32
FP8 = mybir.dt.float8e4
DR = mybir.MatmulPerfMode.DoubleRow
ALU = mybir.AluOpType
ACTF = mybir.ActivationFunctionType

P = 128
D = 1024
N_CORES = 8
RPC = 1024          # rows per core
NRT = RPC // P      # row tiles per core = 8
ST = 2048           # vocab super-tile width
NB = 512            # 512-col group (one PSUM bank)
V0, V1, V2 = 8192, 16384, 25681
PD1, PD2 = 256, 128
B_T = 8192

# windows: (tier0 st, [tier1 sts], [tier2 sts]); within a window each psum
# tile packs groups from different tiers so fills stay balanced vs the
# ScalarE exp+sum drain.
WINDOWS = [
    (0, [0, 1], [0, 1, 2]),
    (1, [2, 3], [3, 4, 5]),
    (2, [4, 5], [6, 7, 8]),
    (3, [6, 7], [9, 10, 11, 12]),
]
GATHER_BLK0 = 8   # first schedule block that may emit a gather/dot

_NC_CACHE = None


def _ceil_div(a, b):
    return (a + b - 1) // b


def _build_graph():
    nc = bacc.Bacc("TRN2", target_bir_lowering=False, debug=False,
                   num_devices=N_CORES)

    ht_ext = nc.declare_dram_parameter("ht", [D, RPC], F32, isOutput=False)
    hr_ext = nc.declare_dram_parameter("hr", [RPC, D], F32, isOutput=False)
    tf_ext = nc.declare_dram_parameter("tf", [P, NRT], F32, isOutput=False)
    wp1_ext = nc.declare_dram_parameter("wp1", [D, PD1], F32, isOutput=False)
    wp2_ext = nc.declare_dram_parameter("wp2", [D, PD2], F32, isOutput=False)
    w0_ext = nc.declare_dram_parameter("w0", [D, V0], F32, isOutput=False)
    w1_ext = nc.declare_dram_parameter("w1", [PD1, V1], F32, isOutput=False)
    w2_ext = nc.declare_dram_parameter("w2", [PD2, V2], F32, isOutput=False)
    wt0_ext = nc.declare_dram_parameter("wt0", [V0, D], F32, isOutput=False)
    wt1_ext = nc.declare_dram_parameter("wt1", [V1, PD1], F32, isOutput=False)
    wt2_ext = nc.declare_dram_parameter("wt2", [V2, PD2], F32, isOutput=False)
    out_ext = nc.declare_dram_parameter("out", [1, 1], F32, isOutput=True)

    with TileContext(nc) as tc:
        with (
            tc.tile_pool(name="res", bufs=1) as res,
            tc.tile_pool(name="w0pool", bufs=2) as w0pool,
            tc.tile_pool(name="w1pool", bufs=4) as w1pool,
            tc.tile_pool(name="w2pool", bufs=6) as w2pool,
            tc.tile_pool(name="hrpool", bufs=2) as hrpool,
            tc.tile_pool(name="expool", bufs=3) as expool,
            tc.tile_pool(name="gpool", bufs=1) as gpool,
            tc.tile_pool(name="prodpool", bufs=1) as prodpool,
            tc.tile_pool(name="psum", bufs=2, space="PSUM") as psum,
        ):
            # ---------------- resident tiles ----------------
            ht8_sb = res.tile([P, 8 * RPC], FP8, tag="ht8")
            wp1_8 = res.tile([P, 8 * PD1], FP8, tag="wp18")
            wp2_8 = res.tile([P, 8 * PD2], FP8, tag="wp28")
            ht_sb = res.tile([P, 8 * RPC], BF16, tag="ht")
            wp1_sb = res.tile([P, 8 * PD1], BF16, tag="wp1")
            wp2_sb = res.tile([P, 8 * PD2], BF16, tag="wp2")
            hp1T_sb = res.tile([P, 2 * RPC], FP8, tag="hp1T")
            hp2T_sb = res.tile([P, 1 * RPC], FP8, tag="hp2T")
            hp1r_sb = res.tile([P, NRT * PD1], F32, tag="hp1r")
            hp2r_sb = res.tile([P, NRT * PD2], F32, tag="hp2r")
            tf_sb = res.tile([P, NRT], F32, tag="tf")
            ge1 = res.tile([P, NRT], F32, tag="ge1")
            ge2 = res.tile([P, NRT], F32, tag="ge2")
            idxf = [res.tile([P, NRT], F32, tag=f"idxf{t}", name=f"idxf{t}")
                    for t in range(3)]
            idxi = [res.tile([P, NRT], I32, tag=f"idxi{t}", name=f"idxi{t}")
                    for t in range(3)]
            tl = [res.tile([P, NRT], F32, tag=f"tl{t}", name=f"tl{t}")
                  for t in range(3)]
            zbig = res.tile([P, NRT * 32], F32, tag="zbig")
            zred = res.tile([P, NRT], F32, tag="zred")
            logz = res.tile([P, NRT], F32, tag="logz")
            d1 = res.tile([P, NRT], F32, tag="d1")
            d2 = res.tile([P, NRT], F32, tag="d2")
            loss8 = res.tile([P, NRT], F32, tag="loss8")
            lossv = res.tile([P, 1], F32, tag="lossv")
            ones = res.tile([P, 1], F32, tag="ones")
            part = res.tile([1, 1], F32, tag="part")

            def load_chunked(dst, src, width):
                nc.gpsimd.dma_start(
                    out=dst[:].rearrange("p (k c) -> p k c", k=8),
                    in_=src[:, :].rearrange("(k p) c -> p k c", p=P))

            # fp8 staging first: these gate the projections and tier0
            load_chunked(ht8_sb, ht_ext, RPC)
            load_chunked(wp1_8, wp1_ext, PD1)
            load_chunked(wp2_8, wp2_ext, PD2)
            nc.sync.dma_start(out=tf_sb[:], in_=tf_ext[:, :])

            nc.vector.memset(zbig[:], 0.0)
            nc.vector.memset(ones[:], 1.0)

            # ---------------- masks and in-tier indices ----------------
            nc.vector.tensor_scalar(out=ge1[:], in0=tf_sb[:], scalar1=float(V0),
                                    scalar2=None, op0=ALU.is_ge)
            nc.vector.tensor_scalar(out=ge2[:], in0=tf_sb[:],
                                    scalar1=float(V0 + V1), scalar2=None,
                                    op0=ALU.is_ge)
            nc.vector.tensor_scalar(out=idxf[0][:], in0=tf_sb[:],
                                    scalar1=float(V0 - 1), scalar2=None,
                                    op0=ALU.min)
            nc.vector.tensor_scalar(out=idxf[1][:], in0=tf_sb[:],
                                    scalar1=-float(V0), scalar2=0.0,
                                    op0=ALU.add, op1=ALU.max)
            nc.vector.tensor_scalar(out=idxf[1][:], in0=idxf[1][:],
                                    scalar1=float(V1 - 1), scalar2=None,
                                    op0=ALU.min)
            nc.vector.tensor_scalar(out=idxf[2][:], in0=tf_sb[:],
                                    scalar1=-float(V0 + V1), scalar2=0.0,
                                    op0=ALU.add, op1=ALU.max)
            nc.vector.tensor_scalar(out=idxf[2][:], in0=idxf[2][:],
                                    scalar1=float(V2 - 1), scalar2=None,
                                    op0=ALU.min)
            for t in range(3):
                nc.vector.tensor_copy(out=idxi[t][:], in_=idxf[t][:])

            # ---------------- fp8 DoubleRow projections (hp1T, hp2T) -------
            ht8v = ht8_sb[:].rearrange("p (k r) -> p k r", k=8)
            wp18v = wp1_8[:].rearrange("p (k c) -> p k c", k=8)
            wp28v = wp2_8[:].rearrange("p (k c) -> p k c", k=8)
            for m in range(PD1 // P):
                for rb in range(RPC // NB):
                    ps = psum.tile([P, ST], F32, tag="ps")
                    for pr in range(4):
                        nc.tensor.matmul(
                            out=ps[:, :NB],
                            lhsT=wp18v[:, 2 * pr: 2 * pr + 2,
                                       m * P:(m + 1) * P],
                            rhs=ht8v[:, 2 * pr: 2 * pr + 2,
                                     rb * NB:(rb + 1) * NB],
                            start=(pr == 0), stop=(pr == 3), perf_mode=DR)
                    nc.vector.tensor_copy(
                        out=hp1T_sb[:, m * RPC + rb * NB:
                                    m * RPC + (rb + 1) * NB],
                        in_=ps[:, :NB])
            for rb in range(RPC // NB):
                ps = psum.tile([P, ST], F32, tag="ps")
                for pr in range(4):
                    nc.tensor.matmul(
                        out=ps[:, :NB],
                        lhsT=wp28v[:, 2 * pr: 2 * pr + 2, 0:P],
                        rhs=ht8v[:, 2 * pr: 2 * pr + 2,
                                 rb * NB:(rb + 1) * NB],
                        start=(pr == 0), stop=(pr == 3), perf_mode=DR)
                nc.vector.tensor_copy(
                    out=hp2T_sb[:, rb * NB:(rb + 1) * NB], in_=ps[:, :NB])

            # ---------------- main stream ----------------
            # (V, K, w_ext, lhsT_sb, wpool, nchunks, wdtype, doublerow)
            tiers = {
                0: (V0, 8, w0_ext, ht8_sb, w0pool, 8, FP8, True),
                1: (V1, 2, w1_ext, hp1T_sb, w1pool, 2, FP8, True),
                2: (V2, 1, w2_ext, hp2T_sb, w2pool, 1, FP8, False),
            }
            gather_src = [wt0_ext, wt1_ext, wt2_ext]
            gdim = [D, PD1, PD2]
            gmax = [V0 - 1, V1 - 1, V2 - 1]
            st_wtile = {}

            def ensure_st(tier, st):
                if (tier, st) in st_wtile:
                    return
                V, K, w_ext, lhsT_sb, wpool, nchunks, wdt, dr = tiers[tier]
                w = min(ST, V - st * ST)
                wtile = wpool.tile([P, nchunks * ST], wdt,
                                   tag=f"w{tier}", name=f"w{tier}")
                for k in range(K):
                    nc.gpsimd.dma_start(
                        out=wtile[:, k * ST: k * ST + w],
                        in_=w_ext[k * P:(k + 1) * P, st * ST: st * ST + w])
                st_wtile[(tier, st)] = wtile

            def emit_rows_proj(rt, t):
                # bf16 rows-orientation projection for the target dot
                pd = PD1 if t == 1 else PD2
                wsrc = wp1_sb if t == 1 else wp2_sb
                dst = hp1r_sb if t == 1 else hp2r_sb
                ps = psum.tile([P, ST], F32, tag="ps")
                for k in range(8):
                    nc.tensor.matmul(
                        out=ps[:, :pd],
                        lhsT=ht_sb[:, k * RPC + rt * P: k * RPC + rt * P + P],
                        rhs=wsrc[:, k * pd:(k + 1) * pd],
                        start=(k == 0), stop=(k == 7))
                nc.vector.tensor_copy(
                    out=dst[:, rt * pd:(rt + 1) * pd], in_=ps[:, :pd])

            def emit_gather_dot(i):
                rt, t = divmod(i, 3)
                if t == 0:
                    hr_t = hrpool.tile([P, D], F32, tag="hrt", name="hrt")
                    nc.sync.dma_start(out=hr_t[:],
                                      in_=hr_ext[rt * P:(rt + 1) * P, :])
                    feat_ap = hr_t[:]
                elif t == 1:
                    emit_rows_proj(rt, 1)
                    feat_ap = hp1r_sb[:, rt * PD1:(rt + 1) * PD1]
                else:
                    emit_rows_proj(rt, 2)
                    feat_ap = hp2r_sb[:, rt * PD2:(rt + 1) * PD2]
                g = gpool.tile([P, gdim[t]], F32, tag=f"g{t}", name=f"g{t}")
                nc.gpsimd.indirect_dma_start(
                    out=g[:], out_offset=None,
                    in_=gather_src[t][:, :],
                    in_offset=IndirectOffsetOnAxis(
                        ap=idxi[t][:, rt:rt + 1], axis=0),
                    bounds_check=gmax[t], oob_is_err=False)
                prod = prodpool.tile([P, D], F32, tag="prod")
                nc.vector.scalar_tensor_tensor(
                    out=prod[:, :gdim[t]],
                    in0=feat_ap, scalar=1.0, in1=g[:],
                    op0=ALU.mult, op1=ALU.mult,
                    accum_out=tl[t][:, rt:rt + 1])

            def emit_tile(groups, rt, zcol):
                ps = psum.tile([P, ST], F32, tag="ps")
                off = 0
                for (tier, st, g, gw) in groups:
                    V, K, w_ext, lhsT_sb, wpool, nchunks, wdt, dr = tiers[tier]
                    wtile = st_wtile[(tier, st)]
                    if dr:
                        lv = lhsT_sb[:].rearrange("p (k r) -> p k r",
                                                  k=nchunks)
                        wv = wtile[:].rearrange("p (k c) -> p k c", k=nchunks)
                        for pr in range(K // 2):
                            nc.tensor.matmul(
                                out=ps[:, off: off + gw],
                                lhsT=lv[:, 2 * pr: 2 * pr + 2,
                                        rt * P: rt * P + P],
                                rhs=wv[:, 2 * pr: 2 * pr + 2,
                                       g * NB: g * NB + gw],
                                start=(pr == 0), stop=(pr == K // 2 - 1),
                                perf_mode=DR)
                    else:
                        for k in range(K):
                            nc.tensor.matmul(
                                out=ps[:, off: off + gw],
                                lhsT=lhsT_sb[:, k * RPC + rt * P:
                                             k * RPC + rt * P + P],
                                rhs=wtile[:, k * ST + g * NB:
                                          k * ST + g * NB + gw],
                                start=(k == 0), stop=(k == K - 1))
                    off += gw
                ex = expool.tile([P, ST], BF16, tag="ex")
                nc.scalar.activation(
                    ex[:, :off], ps[:, :off], ACTF.Exp,
                    accum_out=zbig[:, rt * 32 + zcol: rt * 32 + zcol + 1])

            def st_groups(tier, st):
                V = tiers[tier][0]
                w = min(ST, V - st * ST)
                return [(tier, st, g, min(NB, w - g * NB))
                        for g in range(_ceil_div(w, NB))]

            def build_tiles(As, Bs, Cs):
                # light (B/C-only) tiles first, then the A-bearing tiles
                tiles = []
                na = len(As)
                n_light_b = max(0, len(Bs) - na)
                lb = 0
                ic = 0
                while lb + 2 <= n_light_b:
                    tiles.append(Bs[lb:lb + 2] + Cs[ic:ic + 2])
                    lb += 2; ic += 2
                ib = lb
                for ia in range(na):
                    t = [As[ia]]
                    if ib < len(Bs):
                        t.append(Bs[ib]); ib += 1
                    t += Cs[ic:ic + 2]; ic += 2
                    tiles.append(t)
                while ic < len(Cs):
                    t = Cs[ic:ic + 4]; ic += len(Cs[ic:ic + 4])
                    tiles.append(t)
                return tiles

            zcols = [0] * NRT
            blk = 0
            gi = 0
            for wi, (a_st, b_sts, c_sts) in enumerate(WINDOWS):
                for st in c_sts:
                    ensure_st(2, st)
                for st in b_sts:
                    ensure_st(1, st)
                ensure_st(0, a_st)
                if wi == 0:
                    # bf16 staging for the deferred rows-projections; queued
                    # after window0's W slices so they don't delay the stream
                    load_chunked(ht_sb, ht_ext, RPC)
                    load_chunked(wp1_sb, wp1_ext, PD1)
                    load_chunked(wp2_sb, wp2_ext, PD2)
                As = st_groups(0, a_st)
                Bs = [g for st in b_sts for g in st_groups(1, st)]
                Cs = [g for st in c_sts for g in st_groups(2, st)]
                if wi == 0:
                    # round 1: tier2-only tiles (smallest DMA deps) give the
                    # exp stream an early runway while the big W slices land
                    for rt in range(NRT):
                        for tile_groups in (Cs[0:4], Cs[4:8]):
                            emit_tile(list(tile_groups), rt, zcols[rt])
                            zcols[rt] += 1
                    for rt in range(NRT):
                        for ia in range(4):
                            tile_groups = [As[ia], Bs[2 * ia], Bs[2 * ia + 1],
                                           Cs[8 + ia]]
                            emit_tile(tile_groups, rt, zcols[rt])
                            zcols[rt] += 1
                        blk += 1
                    continue
                for rt in range(NRT):
                    for tile_groups in build_tiles(As, Bs, Cs):
                        emit_tile(tile_groups, rt, zcols[rt])
                        zcols[rt] += 1
                    if blk >= GATHER_BLK0 and gi < 3 * NRT:
                        emit_gather_dot(gi)
                        gi += 1
                    blk += 1
            while gi < 3 * NRT:
                emit_gather_dot(gi)
                gi += 1

            # ---------------- final reduction ----------------
            for rt in range(NRT):
                nc.vector.tensor_reduce(
                    out=zred[:, rt:rt + 1], in_=zbig[:, rt * 32:(rt + 1) * 32],
                    axis=mybir.AxisListType.X, op=ALU.add)
            nc.scalar.activation(logz[:], zred[:], ACTF.Ln)
            # loss8 = logz - (tl0 + ge1*(tl1-tl0) + ge2*(tl2-tl1))
            nc.vector.tensor_tensor(out=d1[:], in0=tl[1][:], in1=tl[0][:],
                                    op=ALU.subtract)
            nc.vector.tensor_tensor(out=d2[:], in0=tl[2][:], in1=tl[1][:],
                                    op=ALU.subtract)
            nc.vector.tensor_tensor(out=d1[:], in0=d1[:], in1=ge1[:],
                                    op=ALU.mult)
            nc.vector.tensor_tensor(out=d2[:], in0=d2[:], in1=ge2[:],
                                    op=ALU.mult)
            nc.vector.tensor_tensor(out=loss8[:], in0=logz[:], in1=tl[0][:],
                                    op=ALU.subtract)
            nc.vector.tensor_tensor(out=loss8[:], in0=loss8[:], in1=d1[:],
                                    op=ALU.subtract)
            nc.vector.tensor_tensor(out=loss8[:], in0=loss8[:], in1=d2[:],
                                    op=ALU.subtract)
            nc.vector.tensor_reduce(out=lossv[:], in_=loss8[:],
                                    axis=mybir.AxisListType.X, op=ALU.add)
            ps = psum.tile([P, ST], F32, tag="ps")
            nc.tensor.matmul(out=ps[0:1, 0:1], lhsT=lossv[:], rhs=ones[:],
                             start=True, stop=True)
            nc.scalar.mul(part[0:1, 0:1], ps[0:1, 0:1], 1.0 / float(B_T))
            nc.sync.dma_start(out=out_ext[:, :], in_=part[:])

    nc.compile()
    return nc


def _get_nc():
    global _NC_CACHE
    if _NC_CACHE is None:
        _NC_CACHE = _build_graph()
    return _NC_CACHE


def _make_in_maps(h, targets, W_head0, W_proj1, W_head1, W_proj2, W_head2):
    h = np.ascontiguousarray(np.asarray(h, dtype=np.float32)).reshape(B_T, D)
    t = np.asarray(targets).reshape(-1).astype(np.float32)
    w0 = np.ascontiguousarray(np.asarray(W_head0, dtype=np.float32))
    w1 = np.ascontiguousarray(np.asarray(W_head1, dtype=np.float32))
    w2 = np.ascontiguousarray(np.asarray(W_head2, dtype=np.float32))
    wp1 = np.ascontiguousarray(np.asarray(W_proj1, dtype=np.float32))
    wp2 = np.ascontiguousarray(np.asarray(W_proj2, dtype=np.float32))
    wt0 = np.ascontiguousarray(w0.T)
    wt1 = np.ascontiguousarray(w1.T)
    wt2 = np.ascontiguousarray(w2.T)

    in_maps = []
    for c in range(N_CORES):
        hc = h[c * RPC:(c + 1) * RPC]
        tc_ = t[c * RPC:(c + 1) * RPC]
        in_maps.append({
            "ht": np.ascontiguousarray(hc.T),
            "hr": hc,
            "tf": np.ascontiguousarray(tc_.reshape(NRT, P).T),
            "wp1": wp1, "wp2": wp2,
            "w0": w0, "w1": w1, "w2": w2,
            "wt0": wt0, "wt1": wt1, "wt2": wt2,
        })
    return in_maps


def kernel(h, targets, token_to_tier, token_to_idx,
           W_head0, W_proj1, W_head1, W_proj2, W_head2):
    in_maps = _make_in_maps(h, targets, W_head0, W_proj1, W_head1,
                            W_proj2, W_head2)
    nc = _get_nc()
    res = run_bass_kernel_spmd(nc, in_maps, core_ids=list(range(N_CORES)))
    total = sum(float(res.results[c]["out"][0, 0]) for c in range(N_CORES))
    return np.float32(total)


# revision 13
# speedup vs baseline: 1.4461x; 1.0016x over previous
"""Adaptive LM head (3-tier chunked softmax cross-entropy) on 8 TRN2 NeuronCores.

Strategy: data-parallel over B_T = 8192 rows (1024 rows/core; weights
replicated). Per core:
  - tier logits via fp8 DoubleRow matmuls (tiers 0/1) and fp8 matmuls
    (tier 2); weights stream from HBM as f32 and are cast in-flight by the
    SWDGE DMA engines. PSUM accumulation over the contraction dim.
  - ScalarE activation(Exp, accum_out=...) fuses exp + per-row sum in a single
    pass over each [128, 2048] logit tile; the schedule packs 512-col groups
    from different tiers into composite tiles and is ACT-bound throughout.
  - target logit = dot(feature_row, W[:, target]) computed in f32/bf16:
    indirect-DMA gather of transposed-weight rows + fused scalar_tensor_tensor
    multiply-reduce, spread through the main stream.
  - per-core partial loss (sum_rows(log Z - target_logit)/8192) is the output;
    the host sums the 8 partials (the unshard step for a DP loss).
"""

import numpy as np

from concourse import bacc, bass, mybir
from concourse.bass import IndirectOffsetOnAxis
from concourse.bass_utils import run_bass_kernel_spmd
from concourse.tile import TileContext

F32 = mybir.dt.float32
BF16 = mybir.dt.bfloat16
I32 = mybir.dt.int32
FP8 = mybir.dt.float8e4
DR = mybir.MatmulPerfMode.DoubleRow
ALU = mybir.AluOpType
ACTF = mybir.ActivationFunctionType

P = 128
D = 1024
N_CORES = 8
RPC = 1024          # rows per core
NRT = RPC // P      # row tiles per core = 8
ST = 2048           # vocab super-tile width
NB = 512            # 512-col group (one PSUM bank)
V0, V1, V2 = 8192, 16384, 25681
PD1, PD2 = 256, 128
B_T = 8192

# windows: (tier0 st, [tier1 sts], [tier2 sts]); within a window each psum
# tile packs groups from different tiers so fills stay balanced vs the
# ScalarE exp+sum drain.
WINDOWS = [
    (0, [0, 1], [0, 1, 2]),
    (1, [2, 3], [3, 4, 5]),
    (2, [4, 5], [6, 7, 8]),
    (3, [6, 7], [9, 10, 11, 12]),
]
GATHER_BLK0 = 8   # first schedule block that may emit a gather/dot

_NC_CACHE = None


def _ceil_div(a, b):
    return (a + b - 1) // b


def _build_graph():
    nc = bacc.Bacc("TRN2", target_bir_lowering=False, debug=False,
                   num_devices=N_CORES)

    ht_ext = nc.declare_dram_parameter("ht", [D, RPC], F32, isOutput=False)
    hr_ext = nc.declare_dram_parameter("hr", [RPC, D], F32, isOutput=False)
    tf_ext = nc.declare_dram_parameter("tf", [P, NRT], F32, isOutput=False)
    wp1_ext = nc.declare_dram_parameter("wp1", [D, PD1], F32, isOutput=False)
    wp2_ext = nc.declare_dram_parameter("wp2", [D, PD2], F32, isOutput=False)
    w0_ext = nc.declare_dram_parameter("w0", [D, V0], F32, isOutput=False)
    w1_ext = nc.declare_dram_parameter("w1", [PD1, V1], F32, isOutput=False)
    w2_ext = nc.declare_dram_parameter("w2", [PD2, V2], F32, isOutput=False)
    wt0_ext = nc.declare_dram_parameter("wt0", [V0, D], F32, isOutput=False)
    wt1_ext = nc.declare_dram_parameter("wt1", [V1, PD1], F32, isOutput=False)
    wt2_ext = nc.declare_dram_parameter("wt2", [V2, PD2], F32, isOutput=False)
    out_ext = nc.declare_dram_parameter("out", [1, 1], F32, isOutput=True)

    with TileContext(nc) as tc:
        with (
            tc.tile_pool(name="res", bufs=1) as res,
            tc.tile_pool(name="w0pool", bufs=2) as w0pool,
            tc.tile_pool(name="w1pool", bufs=4) as w1pool,
            tc.tile_pool(name="w2pool", bufs=6) as w2pool,
            tc.tile_pool(name="hrpool", bufs=2) as hrpool,
            tc.tile_pool(name="expool", bufs=3) as expool,
            tc.tile_pool(name="gpool", bufs=1) as gpool,
            tc.tile_pool(name="prodpool", bufs=1) as prodpool,
            tc.tile_pool(name="psum", bufs=2, space="PSUM") as psum,
        ):
            # ---------------- resident tiles ----------------
            ht8_sb = res.tile([P, 8 * RPC], FP8, tag="ht8")
            wp1_8 = res.tile([P, 8 * PD1], FP8, tag="wp18")
            wp2_8 = res.tile([P, 8 * PD2], FP8, tag="wp28")
            ht_sb = res.tile([P, 8 * RPC], BF16, tag="ht")
            wp1_sb = res.tile([P, 8 * PD1], BF16, tag="wp1")
            wp2_sb = res.tile([P, 8 * PD2], BF16, tag="wp2")
            hp1T_sb = res.tile([P, 2 * RPC], FP8, tag="hp1T")
            hp2T_sb = res.tile([P, 1 * RPC], FP8, tag="hp2T")
            hp1r_sb = res.tile([P, NRT * PD1], F32, tag="hp1r")
            hp2r_sb = res.tile([P, NRT * PD2], F32, tag="hp2r")
            tf_sb = res.tile([P, NRT], F32, tag="tf")
            ge1 = res.tile([P, NRT], F32, tag="ge1")
            ge2 = res.tile([P, NRT], F32, tag="ge2")
            idxf = [res.tile([P, NRT], F32, tag=f"idxf{t}", name=f"idxf{t}")
                    for t in range(3)]
            idxi = [res.tile([P, NRT], I32, tag=f"idxi{t}", name=f"idxi{t}")
                    for t in range(3)]
            tl = [res.tile([P, NRT], F32, tag=f"tl{t}", name=f"tl{t}")
                  for t in range(3)]
            zbig = res.tile([P, NRT * 32], F32, tag="zbig")
            zred = res.tile([P, NRT], F32, tag="zred")
            logz = res.tile([P, NRT], F32, tag="logz")
            d1 = res.tile([P, NRT], F32, tag="d1")
            d2 = res.tile([P, NRT], F32, tag="d2")
            loss8 = res.tile([P, NRT], F32, tag="loss8")
            lossv = res.tile([P, 1], F32, tag="lossv")
            ones = res.tile([P, 1], F32, tag="ones")
            part = res.tile([1, 1], F32, tag="part")

            def load_chunked(dst, src, width):
                nc.gpsimd.dma_start(
                    out=dst[:].rearrange("p (k c) -> p k c", k=8),
                    in_=src[:, :].rearrange("(k p) c -> p k c", p=P))

            # fp8 staging first: these gate the projections and tier0
            load_chunked(ht8_sb, ht_ext, RPC)
            load_chunked(wp1_8, wp1_ext, PD1)
            load_chunked(wp2_8, wp2_ext, PD2)
            nc.sync.dma_start(out=tf_sb[:], in_=tf_ext[:, :])

            nc.vector.memset(zbig[:], 0.0)
            nc.vector.memset(ones[:], 1.0)

            # ---------------- masks and in-tier indices ----------------
            nc.vector.tensor_scalar(out=ge1[:], in0=tf_sb[:], scalar1=float(V0),
                                    scalar2=None, op0=ALU.is_ge)
            nc.vector.tensor_scalar(out=ge2[:], in0=tf_sb[:],
                                    scalar1=float(V0 + V1), scalar2=None,
                                    op0=ALU.is_ge)
            nc.vector.tensor_scalar(out=idxf[0][:], in0=tf_sb[:],
                                    scalar1=float(V0 - 1), scalar2=None,
                                    op0=ALU.min)
            nc.vector.tensor_scalar(out=idxf[1][:], in0=tf_sb[:],
                                    scalar1=-float(V0), scalar2=0.0,
                                    op0=ALU.add, op1=ALU.max)
            nc.vector.tensor_scalar(out=idxf[1][:], in0=idxf[1][:],
                                    scalar1=float(V1 - 1), scalar2=None,
                                    op0=ALU.min)
            nc.vector.tensor_scalar(out=idxf[2][:], in0=tf_sb[:],
                                    scalar1=-float(V0 + V1), scalar2=0.0,
                                    op0=ALU.add, op1=ALU.max)
            nc.vector.tensor_scalar(out=idxf[2][:], in0=idxf[2][:],
                                    scalar1=float(V2 - 1), scalar2=None,
                                    op0=ALU.min)
            for t in range(3):
                nc.vector.tensor_copy(out=idxi[t][:], in_=idxf[t][:])

            # ---------------- fp8 DoubleRow projections (hp1T, hp2T) -------
            ht8v = ht8_sb[:].rearrange("p (k r) -> p k r", k=8)
            wp18v = wp1_8[:].rearrange("p (k c) -> p k c", k=8)
            wp28v = wp2_8[:].rearrange("p (k c) -> p k c", k=8)
            for m in range(PD1 // P):
                for rb in range(RPC // NB):
                    ps = psum.tile([P, ST], F32, tag="ps")
                    for pr in range(4):
                        nc.tensor.matmul(
                            out=ps[:, :NB],
                            lhsT=wp18v[:, 2 * pr: 2 * pr + 2,
                                       m * P:(m + 1) * P],
                            rhs=ht8v[:, 2 * pr: 2 * pr + 2,
                                     rb * NB:(rb + 1) * NB],
                            start=(pr == 0), stop=(pr == 3), perf_mode=DR)
                    nc.vector.tensor_copy(
                        out=hp1T_sb[:, m * RPC + rb * NB:
                                    m * RPC + (rb + 1) * NB],
                        in_=ps[:, :NB])
            for rb in range(RPC // NB):
                ps = psum.tile([P, ST], F32, tag="ps")
                for pr in range(4):
                    nc.tensor.matmul(
                        out=ps[:, :NB],
                        lhsT=wp28v[:, 2 * pr: 2 * pr + 2, 0:P],
                        rhs=ht8v[:, 2 * pr: 2 * pr + 2,
                                 rb * NB:(rb + 1) * NB],
                        start=(pr == 0), stop=(pr == 3), perf_mode=DR)
                nc.vector.tensor_copy(
                    out=hp2T_sb[:, rb * NB:(rb + 1) * NB], in_=ps[:, :NB])

            # ---------------- main stream ----------------
            # (V, K, w_ext, lhsT_sb, wpool, nchunks, wdtype, doublerow)
            tiers = {
                0: (V0, 8, w0_ext, ht8_sb, w0pool, 8, FP8, True),
                1: (V1, 2, w1_ext, hp1T_sb, w1pool, 2, FP8, True),
                2: (V2, 1, w2_ext, hp2T_sb, w2pool, 1, FP8, False),
            }
            gather_src = [wt0_ext, wt1_ext, wt2_ext]
            gdim = [D, PD1, PD2]
            gmax = [V0 - 1, V1 - 1, V2 - 1]
            st_wtile = {}

            def ensure_st(tier, st):
                if (tier, st) in st_wtile:
                    return
                V, K, w_ext, lhsT_sb, wpool, nchunks, wdt, dr = tiers[tier]
                w = min(ST, V - st * ST)
                wtile = wpool.tile([P, nchunks * ST], wdt,
                                   tag=f"w{tier}", name=f"w{tier}")
                for k in range(K):
                    nc.gpsimd.dma_start(
                        out=wtile[:, k * ST: k * ST + w],
                        in_=w_ext[k * P:(k + 1) * P, st * ST: st * ST + w])
                st_wtile[(tier, st)] = wtile

            def emit_rows_proj(rt, t):
                # bf16 rows-orientation projection for the target dot
                pd = PD1 if t == 1 else PD2
                wsrc = wp1_sb if t == 1 else wp2_sb
                dst = hp1r_sb if t == 1 else hp2r_sb
                ps = psum.tile([P, ST], F32, tag="ps")
                for k in range(8):
                    nc.tensor.matmul(
                        out=ps[:, :pd],
                        lhsT=ht_sb[:, k * RPC + rt * P: k * RPC + rt * P + P],
                        rhs=wsrc[:, k * pd:(k + 1) * pd],
                        start=(k == 0), stop=(k == 7))
                nc.vector.tensor_copy(
                    out=dst[:, rt * pd:(rt + 1) * pd], in_=ps[:, :pd])

            def emit_gather_dot(i):
                rt, t = divmod(i, 3)
                if t == 0:
                    hr_t = hrpool.tile([P, D], F32, tag="hrt", name="hrt")
                    nc.sync.dma_start(out=hr_t[:],
                                      in_=hr_ext[rt * P:(rt + 1) * P, :])
                    feat_ap = hr_t[:]
                elif t == 1:
                    emit_rows_proj(rt, 1)
                    feat_ap = hp1r_sb[:, rt * PD1:(rt + 1) * PD1]
                else:
                    emit_rows_proj(rt, 2)
                    feat_ap = hp2r_sb[:, rt * PD2:(rt + 1) * PD2]
                g = gpool.tile([P, gdim[t]], F32, tag=f"g{t}", name=f"g{t}")
                nc.gpsimd.indirect_dma_start(
                    out=g[:], out_offset=None,
                    in_=gather_src[t][:, :],
                    in_offset=IndirectOffsetOnAxis(
                        ap=idxi[t][:, rt:rt + 1], axis=0),
                    bounds_check=gmax[t], oob_is_err=False)
                prod = prodpool.tile([P, D], F32, tag="prod")
                nc.vector.scalar_tensor_tensor(
                    out=prod[:, :gdim[t]],
                    in0=feat_ap, scalar=1.0, in1=g[:],
                    op0=ALU.mult, op1=ALU.mult,
                    accum_out=tl[t][:, rt:rt + 1])

            def emit_tile(groups, rt, zcol):
                ps = psum.tile([P, ST], F32, tag="ps")
                off = 0
                for (tier, st, g, gw) in groups:
                    V, K, w_ext, lhsT_sb, wpool, nchunks, wdt, dr = tiers[tier]
                    wtile = st_wtile[(tier, st)]
                    if dr:
                        lv = lhsT_sb[:].rearrange("p (k r) -> p k r",
                                                  k=nchunks)
                        wv = wtile[:].rearrange("p (k c) -> p k c", k=nchunks)
                        for pr in range(K // 2):
                            nc.tensor.matmul(
                                out=ps[:, off: off + gw],
                                lhsT=lv[:, 2 * pr: 2 * pr + 2,
                                        rt * P: rt * P + P],
                                rhs=wv[:, 2 * pr: 2 * pr + 2,
                                       g * NB: g * NB + gw],
                                start=(pr == 0), stop=(pr == K // 2 - 1),
                                perf_mode=DR)
                    else:
                        for k in range(K):
                            nc.tensor.matmul(
                                out=ps[:, off: off + gw],
                                lhsT=lhsT_sb[:, k * RPC + rt * P:
                                             k * RPC + rt * P + P],
                                rhs=wtile[:, k * ST + g * NB:
                                          k * ST + g * NB + gw],
                                start=(k == 0), stop=(k == K - 1))
                    off += gw
                ex = expool.tile([P, ST], BF16, tag="ex")
                nc.scalar.activation(
                    ex[:, :off], ps[:, :off], ACTF.Exp,
                    accum_out=zbig[:, rt * 32 + zcol: rt * 32 + zcol + 1])

            def st_groups(tier, st):
                V = tiers[tier][0]
                w = min(ST, V - st * ST)
                return [(tier, st, g, min(NB, w - g * NB))
                        for g in range(_ceil_div(w, NB))]

            def build_tiles(As, Bs, Cs):
                # light (B/C-only) tiles first, then the A-bearing tiles
                tiles = []
                na = len(As)
                n_light_b = max(0, len(Bs) - na)
                lb = 0
                ic = 0
                while lb + 2 <= n_light_b:
                    tiles.append(Bs[lb:lb + 2] + Cs[ic:ic + 2])
                    lb += 2; ic += 2
                ib = lb
                for ia in range(na):
                    t = [As[ia]]
                    if ib < len(Bs):
                        t.append(Bs[ib]); ib += 1
                    t += Cs[ic:ic + 2]; ic += 2
                    tiles.append(t)
                while ic < len(Cs):
                    t = Cs[ic:ic + 4]; ic += len(Cs[ic:ic + 4])
                    tiles.append(t)
                return tiles

            zcols = [0] * NRT
            blk = 0
            gi = 0
            for wi, (a_st, b_sts, c_sts) in enumerate(WINDOWS):
                for st in c_sts:
                    ensure_st(2, st)
                if wi == 0:
                    ensure_st(0, a_st)
                for st in b_sts:
                    ensure_st(1, st)
                ensure_st(0, a_st)
                if wi == 0:
                    # bf16 staging for the deferred rows-projections; queued
                    # after window0's W slices so they don't delay the stream
                    load_chunked(ht_sb, ht_ext, RPC)
                    load_chunked(wp1_sb, wp1_ext, PD1)
                    load_chunked(wp2_sb, wp2_ext, PD2)
                As = st_groups(0, a_st)
                Bs = [g for st in b_sts for g in st_groups(1, st)]
                Cs = [g for st in c_sts for g in st_groups(2, st)]
                if wi == 0:
                    # round 1: tier2-only tiles (smallest DMA deps) give the
                    # exp stream an early runway while the big W slices land;
                    # round 2 leads with tier0 tiles (w0 lands before w1)
                    for rt in range(NRT):
                        for tile_groups in (Cs[0:4], Cs[4:8]):
                            emit_tile(list(tile_groups), rt, zcols[rt])
                            zcols[rt] += 1
                    for rt in range(NRT):
                        for tile_groups in (
                            [As[0], As[1], Cs[8], Cs[9]],
                            [As[2], As[3], Cs[10], Cs[11]],
                            Bs[0:4], Bs[4:8],
                        ):
                            emit_tile(list(tile_groups), rt, zcols[rt])
                            zcols[rt] += 1
                        blk += 1
                    continue
                for rt in range(NRT):
                    for tile_groups in build_tiles(As, Bs, Cs):
                        emit_tile(tile_groups, rt, zcols[rt])
                        zcols[rt] += 1
                    if blk >= GATHER_BLK0 and gi < 3 * NRT:
                        emit_gather_dot(gi)
                        gi += 1
                    blk += 1
            while gi < 3 * NRT:
                emit_gather_dot(gi)
                gi += 1

            # ---------------- final reduction ----------------
            for rt in range(NRT):
                nc.vector.tensor_reduce(
                    out=zred[:, rt:rt + 1], in_=zbig[:, rt * 32:(rt + 1) * 32],
                    axis=mybir.AxisListType.X, op=ALU.add)
            nc.scalar.activation(logz[:], zred[:], ACTF.Ln)
            # loss8 = logz - (tl0 + ge1*(tl1-tl0) + ge2*(tl2-tl1))
            nc.vector.tensor_tensor(out=d1[:], in0=tl[1][:], in1=tl[0][:],
                                    op=ALU.subtract)
            nc.vector.tensor_tensor(out=d2[:], in0=tl[2][:], in1=tl[1][:],
                                    op=ALU.subtract)
            nc.vector.tensor_tensor(out=d1[:], in0=d1[:], in1=ge1[:],
                                    op=ALU.mult)
            nc.vector.tensor_tensor(out=d2[:], in0=d2[:], in1=ge2[:],
                                    op=ALU.mult)
            nc.vector.tensor_tensor(out=loss8[:], in0=logz[:], in1=tl[0][:],
                                    op=ALU.subtract)
            nc.vector.tensor_tensor(out=loss8[:], in0=loss8[:], in1=d1[:],
                                    op=ALU.subtract)
            nc.vector.tensor_tensor(out=loss8[:], in0=loss8[:], in1=d2[:],
                                    op=ALU.subtract)
            nc.vector.tensor_reduce(out=lossv[:], in_=loss8[:],
                                    axis=mybir.AxisListType.X, op=ALU.add)
            ps = psum.tile([P, ST], F32, tag="ps")
            nc.tensor.matmul(out=ps[0:1, 0:1], lhsT=lossv[:], rhs=ones[:],
                             start=True, stop=True)
            nc.scalar.mul(part[0:1, 0:1], ps[0:1, 0:1], 1.0 / float(B_T))
            nc.sync.dma_start(out=out_ext[:, :], in_=part[:])

    nc.compile()
    return nc


def _get_nc():
    global _NC_CACHE
    if _NC_CACHE is None:
        _NC_CACHE = _build_graph()
    return _NC_CACHE


def _make_in_maps(h, targets, W_head0, W_proj1, W_head1, W_proj2, W_head2):
    h = np.ascontiguousarray(np.asarray(h, dtype=np.float32)).reshape(B_T, D)
    t = np.asarray(targets).reshape(-1).astype(np.float32)
    w0 = np.ascontiguousarray(np.asarray(W_head0, dtype=np.float32))
    w1 = np.ascontiguousarray(np.asarray(W_head1, dtype=np.float32))
    w2 = np.ascontiguousarray(np.asarray(W_head2, dtype=np.float32))
    wp1 = np.ascontiguousarray(np.asarray(W_proj1, dtype=np.float32))
    wp2 = np.ascontiguousarray(np.asarray(W_proj2, dtype=np.float32))
    wt0 = np.ascontiguousarray(w0.T)
    wt1 = np.ascontiguousarray(w1.T)
    wt2 = np.ascontiguousarray(w2.T)

    in_maps = []
    for c in range(N_CORES):
        hc = h[c * RPC:(c + 1) * RPC]
        tc_ = t[c * RPC:(c + 1) * RPC]
        in_maps.append({
            "ht": np.ascontiguousarray(hc.T),
            "hr": hc,
            "tf": np.ascontiguousarray(tc_.reshape(NRT, P).T),
            "wp1": wp1, "wp2": wp2,
            "w0": w0, "w1": w1, "w2": w2,
            "wt0": wt0, "wt1": wt1, "wt2": wt2,
        })
    return in_maps


def kernel(h, targets, token_to_tier, token_to_idx,
           W_head0, W_proj1, W_head1, W_proj2, W_head2):
    in_maps = _make_in_maps(h, targets, W_head0, W_proj1, W_head1,
                            W_proj2, W_head2)
    nc = _get_nc()
    res = run_bass_kernel_spmd(nc, in_maps, core_ids=list(range(N_CORES)))
    total = sum(float(res.results[c]["out"][0, 0]) for c in range(N_CORES))
    return np.float32(total)


# revision 14
# speedup vs baseline: 1.4494x; 1.0022x over previous
"""Adaptive LM head (3-tier chunked softmax cross-entropy) on 8 TRN2 NeuronCores.

Strategy: data-parallel over B_T = 8192 rows (1024 rows/core; weights
replicated). Per core:
  - tier logits via fp8 DoubleRow matmuls (tiers 0/1) and fp8 matmuls
    (tier 2); weights stream from HBM as f32 and are cast in-flight by the
    SWDGE DMA engines. PSUM accumulation over the contraction dim.
  - ScalarE activation(Exp, accum_out=...) fuses exp + per-row sum in a single
    pass over each [128, 2048] logit tile; the schedule packs 512-col groups
    from different tiers into composite tiles and is ACT-bound throughout.
  - target logit = dot(feature_row, W[:, target]) computed in f32/bf16:
    indirect-DMA gather of transposed-weight rows + fused scalar_tensor_tensor
    multiply-reduce, spread through the main stream.
  - per-core partial loss (sum_rows(log Z - target_logit)/8192) is the output;
    the host sums the 8 partials (the unshard step for a DP loss).
"""

import numpy as np

from concourse import bacc, bass, mybir
from concourse.bass import IndirectOffsetOnAxis
from concourse.bass_utils import run_bass_kernel_spmd
from concourse.tile import TileContext

F32 = mybir.dt.float32
BF16 = mybir.dt.bfloat16
I32 = mybir.dt.int32
FP8 = mybir.dt.float8e4
DR = mybir.MatmulPerfMode.DoubleRow
ALU = mybir.AluOpType
ACTF = mybir.ActivationFunctionType

P = 128
D = 1024
N_CORES = 8
RPC = 1024          # rows per core
NRT = RPC // P      # row tiles per core = 8
ST = 2048           # vocab super-tile width
NB = 512            # 512-col group (one PSUM bank)
V0, V1, V2 = 8192, 16384, 25681
PD1, PD2 = 256, 128
B_T = 8192

# windows: (tier0 st, [tier1 sts], [tier2 sts]); within a window each psum
# tile packs groups from different tiers so fills stay balanced vs the
# ScalarE exp+sum drain.
WINDOWS = [
    (0, [0, 1], [0, 1, 2]),
    (1, [2, 3], [3, 4, 5]),
    (2, [4, 5], [6, 7, 8]),
    (3, [6, 7], [9, 10, 11, 12]),
]
GATHER_BLK0 = 8   # first schedule block that may emit a gather/dot

_NC_CACHE = None


def _ceil_div(a, b):
    return (a + b - 1) // b


def _build_graph():
    nc = bacc.Bacc("TRN2", target_bir_lowering=False, debug=False,
                   num_devices=N_CORES)

    ht_ext = nc.declare_dram_parameter("ht", [D, RPC], F32, isOutput=False)
    hr_ext = nc.declare_dram_parameter("hr", [RPC, D], F32, isOutput=False)
    tf_ext = nc.declare_dram_parameter("tf", [P, NRT], F32, isOutput=False)
    wp1_ext = nc.declare_dram_parameter("wp1", [D, PD1], F32, isOutput=False)
    wp2_ext = nc.declare_dram_parameter("wp2", [D, PD2], F32, isOutput=False)
    w0_ext = nc.declare_dram_parameter("w0", [D, V0], F32, isOutput=False)
    w1_ext = nc.declare_dram_parameter("w1", [PD1, V1], F32, isOutput=False)
    w2_ext = nc.declare_dram_parameter("w2", [PD2, V2], F32, isOutput=False)
    wt0_ext = nc.declare_dram_parameter("wt0", [V0, D], F32, isOutput=False)
    wt1_ext = nc.declare_dram_parameter("wt1", [V1, PD1], F32, isOutput=False)
    wt2_ext = nc.declare_dram_parameter("wt2", [V2, PD2], F32, isOutput=False)
    out_ext = nc.declare_dram_parameter("out", [1, 1], F32, isOutput=True)

    with TileContext(nc) as tc:
        with (
            tc.tile_pool(name="res", bufs=1) as res,
            tc.tile_pool(name="w0pool", bufs=2) as w0pool,
            tc.tile_pool(name="w1pool", bufs=4) as w1pool,
            tc.tile_pool(name="w2pool", bufs=6) as w2pool,
            tc.tile_pool(name="hrpool", bufs=2) as hrpool,
            tc.tile_pool(name="expool", bufs=3) as expool,
            tc.tile_pool(name="gpool", bufs=1) as gpool,
            tc.tile_pool(name="prodpool", bufs=1) as prodpool,
            tc.tile_pool(name="psum", bufs=2, space="PSUM") as psum,
        ):
            # ---------------- resident tiles ----------------
            ht8_sb = res.tile([P, 8 * RPC], FP8, tag="ht8")
            wp1_8 = res.tile([P, 8 * PD1], FP8, tag="wp18")
            wp2_8 = res.tile([P, 8 * PD2], FP8, tag="wp28")
            hp1T_sb = res.tile([P, 2 * RPC], FP8, tag="hp1T")
            hp2T_sb = res.tile([P, 1 * RPC], FP8, tag="hp2T")
            hp1r_sb = res.tile([P, NRT * PD1], F32, tag="hp1r")
            hp2r_sb = res.tile([P, NRT * PD2], F32, tag="hp2r")
            tf_sb = res.tile([P, NRT], F32, tag="tf")
            ge1 = res.tile([P, NRT], F32, tag="ge1")
            ge2 = res.tile([P, NRT], F32, tag="ge2")
            idxf = [res.tile([P, NRT], F32, tag=f"idxf{t}", name=f"idxf{t}")
                    for t in range(3)]
            idxi = [res.tile([P, NRT], I32, tag=f"idxi{t}", name=f"idxi{t}")
                    for t in range(3)]
            tl = [res.tile([P, NRT], F32, tag=f"tl{t}", name=f"tl{t}")
                  for t in range(3)]
            zbig = res.tile([P, NRT * 32], F32, tag="zbig")
            zred = res.tile([P, NRT], F32, tag="zred")
            logz = res.tile([P, NRT], F32, tag="logz")
            d1 = res.tile([P, NRT], F32, tag="d1")
            d2 = res.tile([P, NRT], F32, tag="d2")
            loss8 = res.tile([P, NRT], F32, tag="loss8")
            lossv = res.tile([P, 1], F32, tag="lossv")
            ones = res.tile([P, 1], F32, tag="ones")
            part = res.tile([1, 1], F32, tag="part")

            def load_chunked(dst, src, width):
                nc.gpsimd.dma_start(
                    out=dst[:].rearrange("p (k c) -> p k c", k=8),
                    in_=src[:, :].rearrange("(k p) c -> p k c", p=P))

            # fp8 staging first: these gate the projections and tier0.
            # per-chunk DMAs let the projection matmuls start as chunks land
            for k in range(8):
                nc.gpsimd.dma_start(
                    out=wp2_8[:, k * PD2:(k + 1) * PD2],
                    in_=wp2_ext[k * P:(k + 1) * P, :])
                nc.gpsimd.dma_start(
                    out=wp1_8[:, k * PD1:(k + 1) * PD1],
                    in_=wp1_ext[k * P:(k + 1) * P, :])
                nc.gpsimd.dma_start(
                    out=ht8_sb[:, k * RPC:(k + 1) * RPC],
                    in_=ht_ext[k * P:(k + 1) * P, :])
            nc.sync.dma_start(out=tf_sb[:], in_=tf_ext[:, :])

            nc.vector.memset(zbig[:], 0.0)
            nc.vector.memset(ones[:], 1.0)

            # ---------------- masks and in-tier indices ----------------
            nc.vector.tensor_scalar(out=ge1[:], in0=tf_sb[:], scalar1=float(V0),
                                    scalar2=None, op0=ALU.is_ge)
            nc.vector.tensor_scalar(out=ge2[:], in0=tf_sb[:],
                                    scalar1=float(V0 + V1), scalar2=None,
                                    op0=ALU.is_ge)
            nc.vector.tensor_scalar(out=idxf[0][:], in0=tf_sb[:],
                                    scalar1=float(V0 - 1), scalar2=None,
                                    op0=ALU.min)
            nc.vector.tensor_scalar(out=idxf[1][:], in0=tf_sb[:],
                                    scalar1=-float(V0), scalar2=0.0,
                                    op0=ALU.add, op1=ALU.max)
            nc.vector.tensor_scalar(out=idxf[1][:], in0=idxf[1][:],
                                    scalar1=float(V1 - 1), scalar2=None,
                                    op0=ALU.min)
            nc.vector.tensor_scalar(out=idxf[2][:], in0=tf_sb[:],
                                    scalar1=-float(V0 + V1), scalar2=0.0,
                                    op0=ALU.add, op1=ALU.max)
            nc.vector.tensor_scalar(out=idxf[2][:], in0=idxf[2][:],
                                    scalar1=float(V2 - 1), scalar2=None,
                                    op0=ALU.min)
            for t in range(3):
                nc.vector.tensor_copy(out=idxi[t][:], in_=idxf[t][:])

            # ---------------- fp8 DoubleRow projections (hp1T, hp2T) -------
            ht8v = ht8_sb[:].rearrange("p (k r) -> p k r", k=8)
            wp18v = wp1_8[:].rearrange("p (k c) -> p k c", k=8)
            wp28v = wp2_8[:].rearrange("p (k c) -> p k c", k=8)
            for rb in range(RPC // NB):
                ps = psum.tile([P, ST], F32, tag="ps")
                for pr in range(4):
                    nc.tensor.matmul(
                        out=ps[:, :NB],
                        lhsT=wp28v[:, 2 * pr: 2 * pr + 2, 0:P],
                        rhs=ht8v[:, 2 * pr: 2 * pr + 2,
                                 rb * NB:(rb + 1) * NB],
                        start=(pr == 0), stop=(pr == 3), perf_mode=DR)
                nc.vector.tensor_copy(
                    out=hp2T_sb[:, rb * NB:(rb + 1) * NB], in_=ps[:, :NB])
            for m in range(PD1 // P):
                for rb in range(RPC // NB):
                    ps = psum.tile([P, ST], F32, tag="ps")
                    for pr in range(4):
                        nc.tensor.matmul(
                            out=ps[:, :NB],
                            lhsT=wp18v[:, 2 * pr: 2 * pr + 2,
                                       m * P:(m + 1) * P],
                            rhs=ht8v[:, 2 * pr: 2 * pr + 2,
                                     rb * NB:(rb + 1) * NB],
                            start=(pr == 0), stop=(pr == 3), perf_mode=DR)
                    nc.vector.tensor_copy(
                        out=hp1T_sb[:, m * RPC + rb * NB:
                                    m * RPC + (rb + 1) * NB],
                        in_=ps[:, :NB])

            # ---------------- main stream ----------------
            # (V, K, w_ext, lhsT_sb, wpool, nchunks, wdtype, doublerow)
            tiers = {
                0: (V0, 8, w0_ext, ht8_sb, w0pool, 8, FP8, True),
                1: (V1, 2, w1_ext, hp1T_sb, w1pool, 2, FP8, True),
                2: (V2, 1, w2_ext, hp2T_sb, w2pool, 1, FP8, False),
            }
            gather_src = [wt0_ext, wt1_ext, wt2_ext]
            gdim = [D, PD1, PD2]
            gmax = [V0 - 1, V1 - 1, V2 - 1]
            st_wtile = {}

            def ensure_st(tier, st):
                if (tier, st) in st_wtile:
                    return
                V, K, w_ext, lhsT_sb, wpool, nchunks, wdt, dr = tiers[tier]
                w = min(ST, V - st * ST)
                wtile = wpool.tile([P, nchunks * ST], wdt,
                                   tag=f"w{tier}", name=f"w{tier}")
                for k in range(K):
                    nc.gpsimd.dma_start(
                        out=wtile[:, k * ST: k * ST + w],
                        in_=w_ext[k * P:(k + 1) * P, st * ST: st * ST + w])
                st_wtile[(tier, st)] = wtile

            def emit_rows_proj(rt, t):
                # fp8 DoubleRow rows-orientation projection for the target dot
                pd = PD1 if t == 1 else PD2
                wv = wp18v if t == 1 else wp28v
                dst = hp1r_sb if t == 1 else hp2r_sb
                ps = psum.tile([P, ST], F32, tag="ps")
                for pr in range(4):
                    nc.tensor.matmul(
                        out=ps[:, :pd],
                        lhsT=ht8v[:, 2 * pr: 2 * pr + 2,
                                  rt * P: rt * P + P],
                        rhs=wv[:, 2 * pr: 2 * pr + 2, 0:pd],
                        start=(pr == 0), stop=(pr == 3), perf_mode=DR)
                nc.vector.tensor_copy(
                    out=dst[:, rt * pd:(rt + 1) * pd], in_=ps[:, :pd])

            def emit_gather_dot(i):
                rt, t = divmod(i, 3)
                if t == 0:
                    hr_t = hrpool.tile([P, D], F32, tag="hrt", name="hrt")
                    nc.sync.dma_start(out=hr_t[:],
                                      in_=hr_ext[rt * P:(rt + 1) * P, :])
                    feat_ap = hr_t[:]
                elif t == 1:
                    emit_rows_proj(rt, 1)
                    feat_ap = hp1r_sb[:, rt * PD1:(rt + 1) * PD1]
                else:
                    emit_rows_proj(rt, 2)
                    feat_ap = hp2r_sb[:, rt * PD2:(rt + 1) * PD2]
                g = gpool.tile([P, gdim[t]], F32, tag=f"g{t}", name=f"g{t}")
                nc.gpsimd.indirect_dma_start(
                    out=g[:], out_offset=None,
                    in_=gather_src[t][:, :],
                    in_offset=IndirectOffsetOnAxis(
                        ap=idxi[t][:, rt:rt + 1], axis=0),
                    bounds_check=gmax[t], oob_is_err=False)
                prod = prodpool.tile([P, D], F32, tag="prod")
                nc.vector.scalar_tensor_tensor(
                    out=prod[:, :gdim[t]],
                    in0=feat_ap, scalar=1.0, in1=g[:],
                    op0=ALU.mult, op1=ALU.mult,
                    accum_out=tl[t][:, rt:rt + 1])

            def emit_tile(groups, rt, zcol):
                ps = psum.tile([P, ST], F32, tag="ps")
                off = 0
                for (tier, st, g, gw) in groups:
                    V, K, w_ext, lhsT_sb, wpool, nchunks, wdt, dr = tiers[tier]
                    wtile = st_wtile[(tier, st)]
                    if dr:
                        lv = lhsT_sb[:].rearrange("p (k r) -> p k r",
                                                  k=nchunks)
                        wv = wtile[:].rearrange("p (k c) -> p k c", k=nchunks)
                        for pr in range(K // 2):
                            nc.tensor.matmul(
                                out=ps[:, off: off + gw],
                                lhsT=lv[:, 2 * pr: 2 * pr + 2,
                                        rt * P: rt * P + P],
                                rhs=wv[:, 2 * pr: 2 * pr + 2,
                                       g * NB: g * NB + gw],
                                start=(pr == 0), stop=(pr == K // 2 - 1),
                                perf_mode=DR)
                    else:
                        for k in range(K):
                            nc.tensor.matmul(
                                out=ps[:, off: off + gw],
                                lhsT=lhsT_sb[:, k * RPC + rt * P:
                                             k * RPC + rt * P + P],
                                rhs=wtile[:, k * ST + g * NB:
                                          k * ST + g * NB + gw],
                                start=(k == 0), stop=(k == K - 1))
                    off += gw
                ex = expool.tile([P, ST], BF16, tag="ex")
                nc.scalar.activation(
                    ex[:, :off], ps[:, :off], ACTF.Exp,
                    accum_out=zbig[:, rt * 32 + zcol: rt * 32 + zcol + 1])

            def st_groups(tier, st):
                V = tiers[tier][0]
                w = min(ST, V - st * ST)
                return [(tier, st, g, min(NB, w - g * NB))
                        for g in range(_ceil_div(w, NB))]

            def build_tiles(As, Bs, Cs):
                # light (B/C-only) tiles first, then the A-bearing tiles
                tiles = []
                na = len(As)
                n_light_b = max(0, len(Bs) - na)
                lb = 0
                ic = 0
                while lb + 2 <= n_light_b:
                    tiles.append(Bs[lb:lb + 2] + Cs[ic:ic + 2])
                    lb += 2; ic += 2
                ib = lb
                for ia in range(na):
                    t = [As[ia]]
                    if ib < len(Bs):
                        t.append(Bs[ib]); ib += 1
                    t += Cs[ic:ic + 2]; ic += 2
                    tiles.append(t)
                while ic < len(Cs):
                    t = Cs[ic:ic + 4]; ic += len(Cs[ic:ic + 4])
                    tiles.append(t)
                return tiles

            zcols = [0] * NRT
            blk = 0
            gi = 0
            for wi, (a_st, b_sts, c_sts) in enumerate(WINDOWS):
                for st in c_sts:
                    ensure_st(2, st)
                if wi == 0:
                    ensure_st(0, a_st)
                for st in b_sts:
                    ensure_st(1, st)
                ensure_st(0, a_st)
                As = st_groups(0, a_st)
                Bs = [g for st in b_sts for g in st_groups(1, st)]
                Cs = [g for st in c_sts for g in st_groups(2, st)]
                if wi == 0:
                    # round 1: tier2-only tiles (smallest DMA deps) give the
                    # exp stream an early runway while the big W slices land;
                    # round 2 leads with tier0 tiles (w0 lands before w1)
                    for rt in range(NRT):
                        for tile_groups in (Cs[0:4], Cs[4:8]):
                            emit_tile(list(tile_groups), rt, zcols[rt])
                            zcols[rt] += 1
                    for rt in range(NRT):
                        for tile_groups in (
                            [As[0], As[1], Cs[8], Cs[9]],
                            [As[2], As[3], Cs[10], Cs[11]],
                            Bs[0:4], Bs[4:8],
                        ):
                            emit_tile(list(tile_groups), rt, zcols[rt])
                            zcols[rt] += 1
                        blk += 1
                    continue
                for rt in range(NRT):
                    for tile_groups in build_tiles(As, Bs, Cs):
                        emit_tile(tile_groups, rt, zcols[rt])
                        zcols[rt] += 1
                    if blk >= GATHER_BLK0 and gi < 3 * NRT:
                        emit_gather_dot(gi)
                        gi += 1
                    blk += 1
            while gi < 3 * NRT:
                emit_gather_dot(gi)
                gi += 1

            # ---------------- final reduction ----------------
            for rt in range(NRT):
                nc.vector.tensor_reduce(
                    out=zred[:, rt:rt + 1], in_=zbig[:, rt * 32:(rt + 1) * 32],
                    axis=mybir.AxisListType.X, op=ALU.add)
            nc.scalar.activation(logz[:], zred[:], ACTF.Ln)
            # loss8 = logz - (tl0 + ge1*(tl1-tl0) + ge2*(tl2-tl1))
            nc.vector.tensor_tensor(out=d1[:], in0=tl[1][:], in1=tl[0][:],
                                    op=ALU.subtract)
            nc.vector.tensor_tensor(out=d2[:], in0=tl[2][:], in1=tl[1][:],
                                    op=ALU.subtract)
            nc.vector.tensor_tensor(out=d1[:], in0=d1[:], in1=ge1[:],
                                    op=ALU.mult)
            nc.vector.tensor_tensor(out=d2[:], in0=d2[:], in1=ge2[:],
                                    op=ALU.mult)
            nc.vector.tensor_tensor(out=loss8[:], in0=logz[:], in1=tl[0][:],
                                    op=ALU.subtract)
            nc.vector.tensor_tensor(out=loss8[:], in0=loss8[:], in1=d1[:],
                                    op=ALU.subtract)
            nc.vector.tensor_tensor(out=loss8[:], in0=loss8[:], in1=d2[:],
                                    op=ALU.subtract)
            nc.vector.tensor_reduce(out=lossv[:], in_=loss8[:],
                                    axis=mybir.AxisListType.X, op=ALU.add)
            ps = psum.tile([P, ST], F32, tag="ps")
            nc.tensor.matmul(out=ps[0:1, 0:1], lhsT=lossv[:], rhs=ones[:],
                             start=True, stop=True)
            nc.scalar.mul(part[0:1, 0:1], ps[0:1, 0:1], 1.0 / float(B_T))
            nc.sync.dma_start(out=out_ext[:, :], in_=part[:])

    nc.compile()
    return nc


def _get_nc():
    global _NC_CACHE
    if _NC_CACHE is None:
        _NC_CACHE = _build_graph()
    return _NC_CACHE


def _make_in_maps(h, targets, W_head0, W_proj1, W_head1, W_proj2, W_head2):
    h = np.ascontiguousarray(np.asarray(h, dtype=np.float32)).reshape(B_T, D)
    t = np.asarray(targets).reshape(-1).astype(np.float32)
    w0 = np.ascontiguousarray(np.asarray(W_head0, dtype=np.float32))
    w1 = np.ascontiguousarray(np.asarray(W_head1, dtype=np.float32))
    w2 = np.ascontiguousarray(np.asarray(W_head2, dtype=np.float32))
    wp1 = np.ascontiguousarray(np.asarray(W_proj1, dtype=np.float32))
    wp2 = np.ascontiguousarray(np.asarray(W_proj2, dtype=np.float32))
    wt0 = np.ascontiguousarray(w0.T)
    wt1 = np.ascontiguousarray(w1.T)
    wt2 = np.ascontiguousarray(w2.T)

    in_maps = []
    for c in range(N_CORES):
        hc = h[c * RPC:(c + 1) * RPC]
        tc_ = t[c * RPC:(c + 1) * RPC]
        in_maps.append({
            "ht": np.ascontiguousarray(hc.T),
            "hr": hc,
            "tf": np.ascontiguousarray(tc_.reshape(NRT, P).T),
            "wp1": wp1, "wp2": wp2,
            "w0": w0, "w1": w1, "w2": w2,
            "wt0": wt0, "wt1": wt1, "wt2": wt2,
        })
    return in_maps


def kernel(h, targets, token_to_tier, token_to_idx,
           W_head0, W_proj1, W_head1, W_proj2, W_head2):
    in_maps = _make_in_maps(h, targets, W_head0, W_proj1, W_head1,
                            W_proj2, W_head2)
    nc = _get_nc()
    res = run_bass_kernel_spmd(nc, in_maps, core_ids=list(range(N_CORES)))
    total = sum(float(res.results[c]["out"][0, 0]) for c in range(N_CORES))
    return np.float32(total)
